# revision 1
# baseline (speedup 1.0000x reference)
# Trainium2 Bass kernel for nn_AttentionWithMoLE — 8-core tensor-parallel over heads.
#
# Sharding: core c owns q-heads {2c, 2c+1} (wq cols 128c:128c+128), kv-head c
# (wk/wv cols 64c:64c+64), wo rows 128c:128c+128. Host sums the 8 partial outputs.
#
# MoLE algebra used on device (validated against the reference in numpy):
#   xq = 2*base + sum_e gate_e * lora_e          (softmax gates sum to 1)
#   logits[b,e'] = sum_s r[b,s]*(P1[b,s,e'] - mu[b,s]*C1[s,e']) + C2[e']  (then /tau)
#   P1[b,s,e'] = sum_o base[o]*Wgb[s,o,e'] + sum_er h[er]*W2[s,er,e']
#     where Wgb = sum_e Wgf[s,e,o,e'] (feature-shard) and W2 = Bp-folded Wgf
#     (replicated, /8).  s1 = 4*sum_o base + h.b2 rides as a 5th "e'" column.
#   s2 = sum_f ecat^2 (feature shard), ecat = pl + base broadcast over e.
# Stats ([p,st,b,18]: per proj P1[4]+s1, then s2 x3) AllReduce'd, gates
# computed replicated, then pass B builds q/k/v + RoPE + causal attention
# (scoresT orientation, exp without max-subtraction, bf16 probs, fused rowsum
# via a ones-column appended to V) + output projection (fp16 partials).
import sys
import numpy as np

sys.path.insert(0, '/opt/trn_rl_repo')

import concourse.bass as bass
import concourse.bacc as bacc
import concourse.tile as tile
import concourse.mybir as mybir
from concourse.masks import make_identity

NC = 8
B, S, D, H, KVH, HD, NE, R = 4, 1024, 1024, 16, 8, 64, 4, 16
SCALING, GEPS = 2.0, 1e-6
BS = B * S
NST = S // 128           # 8 s-tiles
NT = NST * B             # 32 token tiles of 128
F16 = mybir.dt.float16
BF16 = mybir.dt.bfloat16
F32 = mybir.dt.float32
AX = mybir.AxisListType
OP = mybir.AluOpType
AF = mybir.ActivationFunctionType

_CACHE = {}
PHASES = 6
APARTS = 7
NOCOLL = False   # timing mode: collectives not supported inside For_i loops


def _bcast_ap(ap, ins):
    """Insert step-0 broadcast dims: ins = list of (pos, count) into ap.ap."""
    dims = [list(d) for d in ap.ap]
    for pos, count in ins:
        dims.insert(pos, [0, count])
    return bass.AP(tensor=ap.tensor, offset=ap.offset, ap=dims)


def _emit(nc, tc, ctx, rep):
    g = nc._kernel_io  # dict of dram handles
    sfx = f"_r{rep}"

    # ---------------- resident pools ----------------
    res = ctx.enter_context(tc.tile_pool(name="res" + sfx, bufs=1))
    pb16_all = res.tile([128, NT, 448], F16)    # [q128|Aq64|k64|Ak64|v64|Av64]
    hqk_all = res.tile([128, NT, 128], F16)     # [er(q0:64,k64:128), t, tok]
    hv_all = res.tile([64, NT, 128], F16)
    xfb_all = res.tile([128, NT, 256], F16)     # 2*base, [q128|k64|v64]
    qT_all = res.tile([64, NST, B, 2, 128], F16)  # [hd, st, b, head, tok] (prescaled 1/8)
    kT_all = res.tile([64, NST, B, 128], F16)
    vaug_all = res.tile([128, NST, B, 65], BF16)
    att_all = res.tile([128, NT, 128], F16)     # [tok(sq), t, o(2 heads)]
    stats_all = res.tile([128, NST, B, 18], F32)
    stats16 = res.tile([128, NST, B, 18], F16)
    stats_ar = res.tile([128, NST, B, 18], F16)
    wqkv_sb = res.tile([128, 8, 451], F16)      # [wq|Aq|wk|Ak|wv|Av|s1cols] per chunk
    blq_sb = res.tile([64, 512], F16)
    blk_sb = res.tile([128, 256], F16)
    blv_sb = res.tile([64, 256], F16)
    bbqk_sb = res.tile([128, 256], F16)
    bbv_sb = res.tile([64, 256], F16)
    wo_sb = res.tile([128, 1024], F16)
    cos_sb = res.tile([128, NST, 96], F32)
    sin_sb = res.tile([128, NST, 96], F32)
    c1_sb = res.tile([128, NST, 12], F32)
    c2it_sb = res.tile([1, 48], F32)
    itau_sb = res.tile([128, 3], F32)
    maskt_sb = res.tile([128, 128], F32)
    ident_sb = res.tile([128, 128], F16)
    ones_sb = res.tile([128, 1], F32)
    geps_sb = res.tile([128, 1], F32)
    zero_sb = res.tile([128, 1], F32)
    gvqk_sb = res.tile([128, B], F32)
    gvv_sb = res.tile([64, B], F32)
    gates_sb = res.tile([1, 48], F16)
    g48_sb = res.tile([48, 1], F16)
    g48x4_sb = res.tile([48, B], F16)
    selqk_sb = res.tile([48, 128], F16)
    selv_sb = res.tile([48, 64], F16)
    bmask_sb = res.tile([48, B], F16)

    dma = nc.sync.dma_start
    dma(out=wqkv_sb[:], in_=g['wqkv16'][:])
    dma(out=blq_sb[:], in_=g['blq16'][:])
    dma(out=blk_sb[64:128, :], in_=g['blk16'][:])
    dma(out=blv_sb[:], in_=g['blv16'][:])
    dma(out=bbqk_sb[:], in_=g['bbqk16'][:])
    dma(out=bbv_sb[:], in_=g['bbv16'][:])
    dma(out=wo_sb[:], in_=g['wo16'][:])
    dma(out=cos_sb[:], in_=g['cosr'][:])
    dma(out=sin_sb[:], in_=g['sinr'][:])
    dma(out=c1_sb[:], in_=g['c1_12'][:])
    c2 = g['c2it48'][:]
    dma(out=c2it_sb[:], in_=bass.AP(tensor=c2.tensor, offset=c2.offset,
                                    ap=[[0, 1], [1, 48]]))
    it = g['itau3'][:]
    dma(out=itau_sb[:], in_=bass.AP(tensor=it.tensor, offset=it.offset,
                                    ap=[[0, 128], [1, 3]]))
    dma(out=maskt_sb[:], in_=g['maskt'][:])
    dma(out=selqk_sb[:], in_=g['selqk'][:])
    dma(out=selv_sb[:], in_=g['selv'][:])
    dma(out=bmask_sb[:], in_=g['bmask'][:])
    make_identity(nc, ident_sb[:])
    nc.vector.memset(ones_sb[:], 1.0)
    nc.vector.memset(geps_sb[:], GEPS)
    nc.vector.memset(zero_sb[:], 0.0)
    nc.vector.memset(vaug_all[:, :, :, 64:65], 1.0)

    # ---------------- phase A ----------------
    if PHASES < 1:
        return
    segs = [(0, 192), (192, 320), (320, 448)]   # per-proj [base|h] blocks
    with tc.tile_pool(name="pa" + sfx, bufs=2) as pa, \
         tc.tile_pool(name="ppb" + sfx, bufs=2, space="PSUM") as ppb, \
         tc.tile_pool(name="ppt" + sfx, bufs=2, space="PSUM") as ppt, \
         tc.tile_pool(name="ppl" + sfx, bufs=2, space="PSUM") as ppl:
        for st in range(NST):
            wcat_t = pa.tile([128, 4, 448], F16, tag="wcat")
            dma(out=wcat_t[:], in_=g['wcat16'][:][st])
            xt = pa.tile([128, B, 8, 128], F16, tag="xt")
            dma(out=xt[:], in_=g['xt16'][:][st])
            for b in range(B):
                t = st * B + b
                pbase = ppb.tile([128, 451], F32, tag="pbase")
                for ch in range(8):
                    nc.tensor.matmul(pbase[:], xt[:, b, ch, :], wqkv_sb[:, ch, :],
                                     start=(ch == 0), stop=(ch == 7))
                pb16 = pb16_all[:, t, :]
                nc.scalar.copy(pb16, pbase[:, 0:448])
                sa = stats_all
                # s1 (exact, via matmul cols) -> stats cols 4, 9, 14
                pbv = pbase[:]
                s1dst = bass.AP(tensor=sa.tensor, offset=sa[:, st, b, 4:5].offset,
                                ap=[list(sa[:, st, b, 4:5].ap[0]), [5, 3]])
                nc.scalar.copy(s1dst, pbase[:, 448:451])
                # er-major h via transposes: hqk [q-h | k-h], hv
                pt = ppt.tile([128, 256], F16, tag="pt")
                nc.tensor.transpose(pt[0:64, 0:128], pb16_all[:, t, 128:192],
                                    ident_sb[:])
                nc.tensor.transpose(pt[64:128, 0:128], pb16_all[:, t, 256:320],
                                    ident_sb[:])
                nc.tensor.transpose(pt[0:64, 128:256], pb16_all[:, t, 384:448],
                                    ident_sb[:])
                nc.scalar.copy(hqk_all[:, t, :], pt[:, 0:128])
                nc.scalar.copy(hv_all[:, t, :], pt[0:64, 128:256])
                if APARTS < 2:
                    continue
                # lora_e via block-diag B' (K=64)
                pl = ppl.tile([128, 1024], F32, tag="pl")
                nc.tensor.matmul(pl[:, 0:512], hqk_all[0:64, t, :], blq_sb[:])
                nc.tensor.matmul(pl[:, 512:768], hqk_all[64:128, t, :], blk_sb[64:128, :])
                nc.tensor.matmul(pl[:, 768:1024], hv_all[:, t, :], blv_sb[:])
                if APARTS < 4:
                    continue
                # ecat = pl + base (broadcast over e); s2 = sum ecat^2 (fused)
                # (DVE reads pl from PSUM directly; Pool is not PSUM-capable)
                ecat = pa.tile([128, 1024], F16, tag="ecat")
                junk = pa.tile([128, 512], F16, tag="junk")
                nc.vector.tensor_add(
                    ecat[:, 0:512].rearrange('p (e o) -> p e o', e=4),
                    pl[:, 0:512].rearrange('p (e o) -> p e o', e=4),
                    _bcast_ap(pb16_all[:, t, 0:128], [(1, 4)]))
                nc.vector.tensor_add(
                    ecat[:, 512:768].rearrange('p (e o) -> p e o', e=4),
                    pl[:, 512:768].rearrange('p (e o) -> p e o', e=4),
                    _bcast_ap(pb16_all[:, t, 192:256], [(1, 4)]))
                nc.vector.tensor_add(
                    ecat[:, 768:1024].rearrange('p (e o) -> p e o', e=4),
                    pl[:, 768:1024].rearrange('p (e o) -> p e o', e=4),
                    _bcast_ap(pb16_all[:, t, 320:384], [(1, 4)]))
                for ci, (f0, f1) in enumerate([(0, 512), (512, 768), (768, 1024)]):
                    nc.scalar.activation(
                        out=junk[:, 0:f1 - f0], in_=ecat[:, f0:f1],
                        func=AF.Square,
                        accum_out=sa[:, st, b, 15 + ci:16 + ci])
                # P1: broadcast mul (Pool) + 3 segmented reduces (DVE)
                wm = pa.tile([128, 4, 448], F16, tag="wm")
                nc.gpsimd.tensor_mul(wm[:], wcat_t[:],
                                     _bcast_ap(pb16_all[:, t, :], [(1, 4)]))
                nc.vector.tensor_reduce(sa[:, st, b, 0:4], wm[:, :, 0:192],
                                        axis=AX.X, op=OP.add)
                nc.vector.tensor_reduce(sa[:, st, b, 5:9], wm[:, :, 192:320],
                                        axis=AX.X, op=OP.add)
                nc.vector.tensor_reduce(sa[:, st, b, 10:14], wm[:, :, 320:448],
                                        axis=AX.X, op=OP.add)

    if PHASES < 2:
        return
    # gate-independent 2*base precompute (overlaps the AllReduce below)
    for t in range(NT):
        nc.vector.tensor_scalar_mul(xfb_all[:, t, 0:128],
                                    pb16_all[:, t, 0:128], 2.0)
        pbv = pb16_all[:, t, 0:1]
        srckv = bass.AP(tensor=pb16_all.tensor, offset=pbv.offset + 192,
                        ap=[list(pbv.ap[0]), [128, 2], [1, 64]])
        nc.vector.tensor_scalar_mul(
            xfb_all[:, t, 128:256].rearrange('p (s o) -> p s o', s=2), srckv, 2.0)
    # ---------------- AllReduce ----------------
    # fp16 collective payload (144KB): stats precision is ample in fp16
    nc.scalar.copy(stats16[:], stats_all[:])
    with tc.tile_pool(name="dr" + sfx, bufs=1, space="DRAM") as dr:
        ar_in = dr.tile([128, NST, B, 18], F16)
        ar_out = dr.tile([128, NST, B, 18], F16)
        dma(out=ar_in[:], in_=stats16[:])
        if NOCOLL:
            dma(out=ar_out[:], in_=ar_in[:])
        else:
            nc.gpsimd.collective_compute(
                "AllReduce", OP.add, replica_groups=[list(range(NC))],
                ins=[ar_in[:]], outs=[ar_out[:]])
        dma(out=stats_ar[:], in_=ar_out[:])

    if PHASES < 3:
        return
    # ---------------- gates (replicated) ----------------
    with tc.tile_pool(name="pg" + sfx, bufs=1) as pg, \
         tc.tile_pool(name="pgp" + sfx, bufs=1, space="PSUM") as pgp:
        lpart = pg.tile([128, 48], F32)
        for pi, Ff in enumerate([NE * H * HD, NE * KVH * HD, NE * KVH * HD]):
            s1 = stats_ar[:, :, :, 5 * pi + 4]
            s2 = stats_ar[:, :, :, 15 + pi]
            mu = pg.tile([128, NST, B], F32, tag=f"mu{pi}")
            vr = pg.tile([128, NST, B], F32, tag=f"vr{pi}")
            rr = pg.tile([128, NST, B], F32, tag=f"rr{pi}")
            rm = pg.tile([128, NST, B], F32, tag=f"rm{pi}")
            t1 = pg.tile([128, NST, B, 4], F32, tag=f"t1{pi}")
            t2 = pg.tile([128, NST, B, 4], F32, tag=f"t2{pi}")
            nc.vector.tensor_scalar_mul(mu[:], s1, 1.0 / Ff)
            nc.vector.tensor_scalar_mul(vr[:], s2, 1.0 / Ff)
            nc.vector.tensor_mul(t2[:, :, :, 0], mu[:], mu[:])
            nc.vector.tensor_sub(vr[:], vr[:], t2[:, :, :, 0])
            nc.scalar.activation(out=rr[:], in_=vr[:], func=AF.Sqrt, bias=geps_sb[:],
                                 scale=1.0)
            nc.vector.reciprocal(rr[:], rr[:])
            nc.vector.tensor_scalar_mul(rr[:], rr[:], itau_sb[:, pi:pi + 1])
            nc.vector.tensor_mul(rm[:], rr[:], mu[:])
            # all 4 experts at once: t1 = P1*rr - C1*rm, reduced over st
            nc.vector.tensor_mul(t1[:], stats_ar[:, :, :, 5 * pi:5 * pi + 4],
                                 _bcast_ap(rr[:], [(3, 4)]))
            nc.vector.tensor_mul(t2[:], _bcast_ap(c1_sb[:, :, 4 * pi:4 * pi + 4],
                                                  [(2, B)]),
                                 _bcast_ap(rm[:], [(3, 4)]))
            nc.vector.tensor_sub(t1[:], t1[:], t2[:])
            src = bass.AP(tensor=t1.tensor, offset=t1[:].offset,
                          ap=[list(t1[:].ap[0]), [4, B], [1, 4], [4 * B, NST]])
            dst = bass.AP(tensor=lpart.tensor, offset=lpart[:].offset + 16 * pi,
                          ap=[list(lpart[:].ap[0]), [4, B], [1, 4]])
            nc.vector.tensor_reduce(dst, src, axis=AX.X, op=OP.add)
        pl48 = pgp.tile([1, 48], F32)
        nc.tensor.matmul(pl48[:], ones_sb[:], lpart[:])
        lg = pg.tile([1, 48], F32)
        nc.vector.tensor_add(lg[:], pl48[:], c2it_sb[:])
        # softmax over e' in groups of 4 (layout (p, b, e'))
        mx = pg.tile([1, 12], F32)
        nc.vector.tensor_reduce(mx[:], lg[:].rearrange('o (g e) -> o g e', e=4),
                                axis=AX.X, op=OP.max)
        for e in range(4):
            nc.vector.tensor_sub(lg[:].rearrange('o (g e) -> o g e', e=4)[:, :, e],
                                 lg[:].rearrange('o (g e) -> o g e', e=4)[:, :, e],
                                 mx[:])
        nc.scalar.activation(out=lg[:], in_=lg[:], func=AF.Exp)
        sm = pg.tile([1, 12], F32)
        nc.vector.tensor_reduce(sm[:], lg[:].rearrange('o (g e) -> o g e', e=4),
                                axis=AX.X, op=OP.add)
        nc.vector.reciprocal(sm[:], sm[:])
        for e in range(4):
            nc.vector.tensor_mul(gates_sb[:].rearrange('o (g e) -> o g e', e=4)[:, :, e],
                                 lg[:].rearrange('o (g e) -> o g e', e=4)[:, :, e],
                                 sm[:])
        # broadcast gates to per-er rows: transpose to 48 partitions, expand
        # per-b columns via mask, then one-hot selection matmuls.
        g48T = pgp.tile([48, 1], F16)
        nc.tensor.transpose(g48T[:], gates_sb[:], ident_sb[0:1, 0:1])
        nc.scalar.copy(g48_sb[:], g48T[:])
        nc.vector.tensor_mul(g48x4_sb[:], bmask_sb[:],
                             _bcast_ap(g48_sb[:, 0], [(1, B)]))
        gvp = pgp.tile([128, B], F32)
        gvvp = pgp.tile([64, B], F32)
        nc.tensor.matmul(gvp[:], selqk_sb[:], g48x4_sb[:])
        nc.tensor.matmul(gvvp[:], selv_sb[:], g48x4_sb[:])
        nc.scalar.copy(gvqk_sb[:], gvp[:])
        nc.scalar.copy(gvv_sb[:], gvvp[:])

    if PHASES < 4:
        return
    # ---------------- phase B (batched over all 8 s-tiles per b) ----------------
    with tc.tile_pool(name="pb" + sfx, bufs=2) as pb, \
         tc.tile_pool(name="ppx" + sfx, bufs=1, space="PSUM") as ppx, \
         tc.tile_pool(name="ppt2" + sfx, bufs=1, space="PSUM") as ppt2:
        for b in range(B):
            # gate-scaled h for all 8 st at once (t stride for fixed b = B*128)
            hgqk = pb.tile([128, NST, 128], F16, tag="hgqk")
            hgv = pb.tile([64, NST, 128], F16, tag="hgv")
            hq0 = hqk_all[:, b, :]
            nc.gpsimd.tensor_scalar_mul(
                hgqk[:], bass.AP(tensor=hqk_all.tensor, offset=hq0.offset,
                                 ap=[list(hq0.ap[0]), [B * 128, NST], [1, 128]]),
                gvqk_sb[:, b:b + 1])
            hv0 = hv_all[0:64, b, :]
            nc.gpsimd.tensor_scalar_mul(
                hgv[:], bass.AP(tensor=hv_all.tensor, offset=hv0.offset,
                                ap=[list(hv0.ap[0]), [B * 128, NST], [1, 128]]),
                gvv_sb[:, b:b + 1])
            px = ppx.tile([128, NST, 256], F32, tag="px")
            for st in range(NST):
                nc.tensor.matmul(px[:, st, :], hgqk[:, st, :], bbqk_sb[:],
                                 start=True, stop=False)
                nc.tensor.matmul(px[:, st, :], hgv[:, st, :], bbv_sb[:],
                                 start=False, stop=True)
            xf = pb.tile([128, NST, 192], F16, tag="xf")
            xfb0 = xfb_all[:, b, :]
            xfbv = bass.AP(tensor=xfb_all.tensor, offset=xfb0.offset,
                           ap=[list(xfb0.ap[0]), [B * 256, NST], [1, 192]])
            nc.vector.tensor_add(
                xf[:], xfbv,
                px[:].rearrange('p st o -> p st o')[:, :, 0:192])
            xfbv2 = bass.AP(tensor=xfb_all.tensor, offset=xfb0.offset + 192,
                            ap=[list(xfb0.ap[0]), [B * 256, NST], [1, 64]])
            nc.vector.tensor_add(
                vaug_all[:, :, b, 0:64], xfbv2,
                px[:].rearrange('p st o -> p st o')[:, :, 192:256])
            # rope for all st: 4-dim APs [p, st, head(3), 32]
            xr = pb.tile([128, NST, 192], F16, tag="xr")
            tmp1 = pb.tile([128, NST, 96], F32, tag="tmp1")
            tmp2 = pb.tile([128, NST, 96], F32, tag="tmp2")
            tmp3 = pb.tile([128, NST, 96], F32, tag="tmp3")
            tmp4 = pb.tile([128, NST, 96], F32, tag="tmp4")
            xfo = xf[:]
            pstep = xfo.ap[0][0]
            xe = bass.AP(tensor=xf.tensor, offset=xfo.offset,
                         ap=[[pstep, 128], [192, NST], [64, 3], [2, 32]])
            xo = bass.AP(tensor=xf.tensor, offset=xfo.offset + 1,
                         ap=[[pstep, 128], [192, NST], [64, 3], [2, 32]])
            xro = xr[:]
            prstep = xro.ap[0][0]
            xre = bass.AP(tensor=xr.tensor, offset=xro.offset,
                          ap=[[prstep, 128], [192, NST], [64, 3], [2, 32]])
            xroo = bass.AP(tensor=xr.tensor, offset=xro.offset + 1,
                           ap=[[prstep, 128], [192, NST], [64, 3], [2, 32]])
            cb = cos_sb[:].rearrange('p st (h i) -> p st h i', h=3)
            sb_ = sin_sb[:].rearrange('p st (h i) -> p st h i', h=3)
            T1 = tmp1[:].rearrange('p st (h i) -> p st h i', h=3)
            T2 = tmp2[:].rearrange('p st (h i) -> p st h i', h=3)
            T3 = tmp3[:].rearrange('p st (h i) -> p st h i', h=3)
            T4 = tmp4[:].rearrange('p st (h i) -> p st h i', h=3)
            nc.vector.tensor_mul(T1, xe, cb)
            nc.vector.tensor_mul(T2, xo, sb_)
            nc.vector.tensor_sub(xre, T1, T2)
            nc.gpsimd.tensor_mul(T3, xe, sb_)
            nc.gpsimd.tensor_mul(T4, xo, cb)
            nc.gpsimd.tensor_add(xroo, T3, T4)
            # transposes into a per-b psum strip (512-col stride keeps each
            # 384-wide transpose inside a psum bank)
            pt = ppt2.tile([64, NST, 512], F16, tag="pt")
            for st in range(NST):
                nc.tensor.transpose(pt[:, st, 0:128], xr[:, st, 0:64], ident_sb[:])
                nc.tensor.transpose(pt[:, st, 128:256], xr[:, st, 64:128],
                                    ident_sb[:])
                nc.tensor.transpose(pt[:, st, 256:384], xr[:, st, 128:192],
                                    ident_sb[:])
            qd0 = qT_all[:, 0, b, :, :]
            nc.scalar.mul(
                bass.AP(tensor=qT_all.tensor, offset=qd0.offset,
                        ap=[list(qd0.ap[0]), [B * 256, NST], [1, 256]]),
                pt[:, :, 0:256], 0.125)
            kd0 = kT_all[:, 0, b, :]
            nc.scalar.copy(
                bass.AP(tensor=kT_all.tensor, offset=kd0.offset,
                        ap=[list(kd0.ap[0]), [B * 128, NST], [1, 128]]),
                pt[:, :, 256:384])

    if PHASES < 5:
        return
    # ---------------- attention ----------------
    with tc.tile_pool(name="pat" + sfx, bufs=2) as pat, \
         tc.tile_pool(name="pps" + sfx, bufs=2, space="PSUM") as pps, \
         tc.tile_pool(name="ppa" + sfx, bufs=2, space="PSUM") as ppa:
        for b in range(B):
            for hh in range(2):
                probs = pat.tile([128, NST, 1024], BF16, tag="probs")
                for i in range(NST):
                    ki = kT_all[:, i, b, :]
                    c0 = i * 128
                    ps = pps.tile([128, 1024], F32, tag="ps")
                    for (m0, m1) in [(c0, 512), (max(512, c0), 1024)]:
                        if m0 >= m1:
                            continue
                        st0 = m0 // 128
                        rhs = qT_all[:, st0:(m1 // 128), b, hh, :]
                        nc.tensor.matmul(ps[:, m0:m1], ki, rhs)
                    nc.vector.tensor_add(ps[:, c0:c0 + 128], ps[:, c0:c0 + 128],
                                         maskt_sb[:])
                    nc.scalar.activation(out=probs[:, i, c0:1024],
                                         in_=ps[:, c0:1024], func=AF.Exp)
                pa8 = ppa.tile([128, NST, 128], F32, tag="pa8")
                for j in range(NST):
                    for i in range(j + 1):
                        nc.tensor.matmul(pa8[:, j, 0:65],
                                         probs[:, i, 128 * j:128 * j + 128],
                                         vaug_all[:, i, b, :],
                                         start=(i == 0), stop=(i == j))
                rc8 = pat.tile([128, NST], F32, tag="rc8")
                pav = pa8[:]
                nc.vector.reciprocal(
                    rc8[:], bass.AP(tensor=pa8.tensor, offset=pav.offset + 64,
                                    ap=[list(pav.ap[0]), [128, NST]]))
                ad0 = att_all[:, b, 64 * hh:64 * hh + 64]
                nc.vector.tensor_mul(
                    bass.AP(tensor=att_all.tensor, offset=ad0.offset,
                            ap=[list(ad0.ap[0]), [B * 128, NST], [1, 64]]),
                    pa8[:, :, 0:64], _bcast_ap(rc8[:], [(2, 64)]))

    if PHASES < 6:
        return
    # ---------------- output projection ----------------
    with tc.tile_pool(name="pw" + sfx, bufs=2) as pw, \
         tc.tile_pool(name="ppo" + sfx, bufs=2, space="PSUM") as ppo, \
         tc.tile_pool(name="ppat" + sfx, bufs=2, space="PSUM") as ppat:
        for b in range(B):
            paT = ppat.tile([128, NST, 128], F16, tag="paT")
            for st in range(NST):
                nc.tensor.transpose(paT[:, st, :], att_all[:, st * B + b, :],
                                    ident_sb[:])
            aT = pw.tile([128, NST, 128], F16, tag="aT")
            nc.scalar.copy(aT[:], paT[:])
            ob = pw.tile([128, NST, 1024], F16, tag="ob")
            for st in range(NST):
                po = ppo.tile([128, 1024], F32, tag="po")
                nc.tensor.matmul(po[:, 0:512], aT[:, st, :], wo_sb[:, 0:512])
                nc.tensor.matmul(po[:, 512:1024], aT[:, st, :], wo_sb[:, 512:1024])
                if st % 2 == 0:
                    nc.scalar.copy(ob[:, st, :], po[:])
                else:
                    nc.vector.tensor_copy(ob[:, st, :], po[:])
            dma(out=g['outp'][:][b], in_=ob[:])


def build_kernel(repeat=1, loopn=0):
    key = (repeat, PHASES, APARTS, loopn, NOCOLL)
    if key in _CACHE:
        return _CACHE[key]
    nc = bacc.Bacc()
    io = {}
    def din(name, shape, dt):
        io[name] = nc.dram_tensor(name, list(shape), dt, kind="ExternalInput")
    din('xt16', (NST, 128, B, 8, 128), F16)
    din('wqkv16', (128, 8, 451), F16)
    din('wcat16', (NST, 128, 4, 448), F16)
    din('blq16', (64, 512), F16)
    din('blk16', (64, 256), F16)
    din('blv16', (64, 256), F16)
    din('bbqk16', (128, 256), F16)
    din('bbv16', (64, 256), F16)
    din('c1_12', (128, NST, 12), F32)
    din('c2it48', (48,), F32)
    din('itau3', (3,), F32)
    din('cosr', (128, NST, 96), F32)
    din('sinr', (128, NST, 96), F32)
    din('maskt', (128, 128), F32)
    din('selqk', (48, 128), F16)
    din('selv', (48, 64), F16)
    din('bmask', (48, B), F16)
    din('wo16', (128, 1024), F16)
    io['outp'] = nc.dram_tensor('outp', [B, 128, NST, 1024], F16,
                                kind="ExternalOutput")
    nc._kernel_io = io
    from contextlib import ExitStack
    with tile.TileContext(nc) as tc:
        if loopn:
            # hardware loop: same body executed loopn times (timing mode)
            with tc.For_i(0, loopn):
                with ExitStack() as ctx:
                    _emit(nc, tc, ctx, 0)
        else:
            for rep in range(repeat):
                with ExitStack() as ctx:
                    _emit(nc, tc, ctx, rep)
    nc.finalize()
    _CACHE[key] = nc
    return nc


def prep_inputs(inputs):
    """Host-side sharding prep: returns in_maps (list of 8 dicts)."""
    f = np.float32
    x = np.asarray(inputs['x'], f)
    # xt16[st, p, b, ch, tk] = x[b, st*128+tk, ch*128+p]
    xr8 = np.asarray(x.transpose(2, 0, 1), np.float16).reshape(8, 128, B, NST, 128)
    xt16 = np.ascontiguousarray(xr8.transpose(3, 1, 2, 0, 4))
    cos3 = np.tile(np.asarray(inputs['cos'], f), (1, 3)).reshape(NST, 128, 96)
    sin3 = np.tile(np.asarray(inputs['sin'], f), (1, 3)).reshape(NST, 128, 96)
    cosr = np.ascontiguousarray(cos3.transpose(1, 0, 2))
    sinr = np.ascontiguousarray(sin3.transpose(1, 0, 2))
    maskt = np.ascontiguousarray(np.asarray(inputs['mask'], f)[0:128, 0:128].T)
    # gate broadcast helpers: gates flat col = pi*16 + b*4 + e
    selqk = np.zeros((48, 128), np.float16)
    for er in range(128):
        pi, e = er // 64, (er % 64) // 16
        for b in range(B):
            selqk[pi * 16 + b * 4 + e, er] = 1.0
    selv = np.zeros((48, 64), np.float16)
    for er in range(64):
        for b in range(B):
            selv[2 * 16 + b * 4 + er // 16, er] = 1.0
    bmask = np.zeros((48, B), np.float16)
    for fl in range(48):
        bmask[fl, (fl % 16) // 4] = 1.0
    in_maps = []
    pr = {}
    for p, Of in [('q', H * HD), ('k', KVH * HD), ('v', KVH * HD)]:
        A = np.asarray(inputs[f'A_{p}'], f)
        Bm = np.asarray(inputs[f'B_{p}'], f)
        gg = np.asarray(inputs[f'g_{p}'], f)
        bb = np.asarray(inputs[f'b_{p}'], f)
        We = np.asarray(inputs[f'We_{p}'], f)
        tau = float(np.asarray(inputs[f'tau_{p}']))
        itau = 1.0 / max(tau, 1e-6)
        Acat = np.ascontiguousarray(A.transpose(1, 0, 2).reshape(D, NE * R))
        Bp = SCALING * Bm                      # [E,R,Of]
        gv = gg.reshape(NE, Of)
        Wgf = We.reshape(S, NE, Of, NE) * gv[None, :, :, None]
        Wgb = Wgf.sum(axis=1)                  # [S, Of, 4]
        W2 = np.einsum('ero,seoE->serE', Bp, Wgf).reshape(S, NE * R, NE) / NC
        b2 = Bp.sum(axis=2).reshape(NE * R) / NC
        C1 = Wgf.sum(axis=(1, 2)) * itau       # [S,4]
        C2 = (We.reshape(S, NE * Of, NE) * bb[None, :, None]).sum((0, 1)) * itau
        pr[p] = dict(Acat=Acat, Bp=Bp, Wgb=Wgb, W2=W2, b2=b2, C1=C1, C2=C2,
                     itau=itau)
    c1_12 = np.ascontiguousarray(
        np.concatenate([pr[p]['C1'] for p in 'qkv'], 1).astype(f)
        .reshape(NST, 128, 12).transpose(1, 0, 2))
    c2it48 = np.zeros(48, f)
    for pi, p in enumerate('qkv'):
        for b in range(B):
            c2it48[pi * 16 + b * 4:pi * 16 + b * 4 + 4] = pr[p]['C2']
    itau3 = np.array([pr[p]['itau'] for p in 'qkv'], f)

    wq = np.asarray(inputs['wq'], f)
    wk = np.asarray(inputs['wk'], f)
    wv = np.asarray(inputs['wv'], f)
    wo = np.asarray(inputs['wo'], f)
    # s1 columns (exact full-feature sums, /NC since every core computes them)
    s1cols = np.stack(
        [(4.0 * Wfull.sum(axis=1) / NC + pr[p]['Acat'] @ pr[p]['b2'])
         for p, Wfull in [('q', wq), ('k', wk), ('v', wv)]], axis=1)  # [D,3]
    for c in range(NC):
        qs = slice(128 * c, 128 * c + 128)
        ks = slice(64 * c, 64 * c + 64)
        # rhs chunks: [wq(128)|Aq(64)|wk(64)|Ak(64)|wv(64)|Av(64)|s1(3)] per ch
        wqkv = np.concatenate(
            [wq[:, qs], pr['q']['Acat'], wk[:, ks], pr['k']['Acat'],
             wv[:, ks], pr['v']['Acat'], s1cols], 1)             # [D,451]
        wqkv16 = np.ascontiguousarray(
            wqkv.reshape(8, 128, 451).transpose(1, 0, 2)).astype(np.float16)
        # wcat [S -> (st,p), 4, 448]
        wcat = np.zeros((S, 4, 448), f)
        for pi_, (p, sh, o0) in enumerate([('q', qs, 0), ('k', ks, 192),
                                           ('v', ks, 320)]):
            wcat[:, :, o0:o0 + (128 if p == 'q' else 64)] = \
                pr[p]['Wgb'][:, sh, :].transpose(0, 2, 1)
            h0 = o0 + (128 if p == 'q' else 64)
            wcat[:, :, h0:h0 + 64] = pr[p]['W2'].transpose(0, 2, 1)
        wcat16 = np.ascontiguousarray(
            wcat.reshape(NST, 128, 4, 448)).astype(np.float16)
        m = dict(xt16=xt16, wqkv16=wqkv16, wcat16=wcat16, c1_12=c1_12,
                 c2it48=c2it48, itau3=itau3, cosr=cosr, sinr=sinr, maskt=maskt,
                 selqk=selqk, selv=selv, bmask=bmask)
        # lora block matrices
        blq = np.zeros((64, 512), f)
        blk = np.zeros((64, 256), f)
        blv = np.zeros((64, 256), f)
        for e in range(NE):
            blq[e * 16:e * 16 + 16, e * 128:e * 128 + 128] = pr['q']['Bp'][e][:, qs]
            blk[e * 16:e * 16 + 16, e * 64:e * 64 + 64] = pr['k']['Bp'][e][:, ks]
            blv[e * 16:e * 16 + 16, e * 64:e * 64 + 64] = pr['v']['Bp'][e][:, ks]
        bbqk = np.zeros((128, 256), f)
        bbv = np.zeros((64, 256), f)
        bbqk[0:64, 0:128] = pr['q']['Bp'][:, :, qs].reshape(64, 128)
        bbqk[64:128, 128:192] = pr['k']['Bp'][:, :, ks].reshape(64, 64)
        bbv[:, 192:256] = pr['v']['Bp'][:, :, ks].reshape(64, 64)
        m['blq16'] = blq.astype(np.float16)
        m['blk16'] = blk.astype(np.float16)
        m['blv16'] = blv.astype(np.float16)
        m['bbqk16'] = bbqk.astype(np.float16)
        m['bbv16'] = bbv.astype(np.float16)
        m['wo16'] = wo[qs, :].astype(np.float16)
        in_maps.append(m)
    return in_maps


def run_on_device(in_maps, repeat=1, loopn=0):
    from concourse.bass_utils import run_bass_kernel_spmd
    nc = build_kernel(repeat, loopn)
    res = run_bass_kernel_spmd(nc, in_maps, list(range(NC)))
    return res


def _run_sim(in_maps):
    from concourse.bass_interp import MultiCoreSim
    nc = build_kernel(1)
    sim = MultiCoreSim(nc, NC, num_workers=NC)
    for c in range(NC):
        for name, arr in in_maps[c].items():
            sim.cores[c].tensor(name)[:] = arr
    sim.simulate()
    return [{'outp': np.asarray(sim.cores[c].tensor('outp'))} for c in range(NC)]


def kernel(**inputs):
    in_maps = prep_inputs(inputs)
    try:
        results = run_on_device(in_maps, repeat=1).results
    except Exception as e:
        sys.stderr.write(f"device run failed ({e}); falling back to CoreSim\n")
        results = _run_sim(in_maps)
    out = np.zeros((B, 128, NST, 1024), np.float32)
    for c in range(NC):
        out += np.asarray(results[c]['outp'], np.float32)
    return np.ascontiguousarray(out.transpose(0, 2, 1, 3)).reshape(B, S, 1024)



# revision 3
# speedup vs baseline: 1.4786x; 1.4786x over previous
# Trainium2 Bass kernel v2 for nn_AttentionWithMoLE — 8-core TP over heads.
#
# v2 redesign: phase-A GEMM emitted TRANSPOSED ([feature, token] rows) so the
# MoLE stats ride the PE via gram/diagonal matmuls instead of Pool/DVE/Act
# elementwise sweeps:
#   bcat chunks (per st, all 4 b batched, N=512):
#     ch0 = base_q(128) ; ch1 = [h_q 64 | s1 3 | pad] ; ch2 = [base_k | h_k] ;
#     ch3 = [base_v | h_v]
#   P1[tok,e'] = diag of gram(wcatT_e', bcat)  (per-position weights as lhsT)
#   s2 = diag of gram(bcat_b, w_b) with w = [E*base+2u ; G h] built by 5 PE
#     matmuls from host-folded bilinear forms (u = Bflat^T h, G = blockdiag BB^T)
#   s1 = GEMM columns, tiny PE transposes into stats layout.
# Phase B builds xq/xk/xv transposed via PE (base/lora/permuted-rope weight
# folding), RoPE = C (.) x + S (.) (P x) on DVE with host tables; k rows are
# duplicated into both partition bands so both q heads find their k in-band.
# Attention/out-projection follow the v1 scheme (scoresT, exp, fused rowsum
# via ones-column, recip-normalize, wo with host-summed partials).
import sys
import numpy as np

sys.path.insert(0, '/opt/trn_rl_repo')

import concourse.bass as bass
import concourse.bacc as bacc
import concourse.tile as tile
import concourse.mybir as mybir
from concourse.masks import make_identity

NC = 8
B, S, D, H, KVH, HD, NE, R = 4, 1024, 1024, 16, 8, 64, 4, 16
SCALING, GEPS = 2.0, 1e-6
NST = S // 128           # 8 s-tiles
NT = NST * B             # 32 token tiles of 128
F16 = mybir.dt.float16
BF16 = mybir.dt.bfloat16
F32 = mybir.dt.float32
AX = mybir.AxisListType
OP = mybir.AluOpType
AF = mybir.ActivationFunctionType

_CACHE = {}
NOCOLL = False   # timing mode: collectives not supported inside For_i loops
PHASES = 9


def _ap(base, doff, dims):
    """AP with base's partition dim, extra element offset doff, free dims."""
    return bass.AP(tensor=base.tensor, offset=base.offset + doff,
                   ap=[list(base.ap[0])] + [list(d) for d in dims])


def _bcast_ap(ap, ins):
    dims = [list(d) for d in ap.ap]
    for pos, count in ins:
        dims.insert(pos, [0, count])
    return bass.AP(tensor=ap.tensor, offset=ap.offset, ap=dims)


def _emit(nc, tc, ctx, rep):
    g = nc._kernel_io
    sfx = f"_r{rep}"
    dma = nc.sync.dma_start

    # ---------------- resident pool ----------------
    res = ctx.enter_context(tc.tile_pool(name="res" + sfx, bufs=1))
    wqkv_sb = res.tile([128, 8, 4, 128], F16)
    wws_sb = res.tile([128, 5, 128], F16)
    wpb_sb = res.tile([128, 10, 128], F16)
    selsum_sb = res.tile([128, 24, 18], F16)
    rope_sb = res.tile([128, 4, NST, 128], F16)
    bcat_all = res.tile([128, NST, 4, B, 128], F16)
    stats_all = res.tile([128, NST, B, 18], F16)
    stats_ar = res.tile([128, NST, B, 18], F16)
    xrq_all = res.tile([128, B, NST, 128], F16)
    xrkv_all = res.tile([128, B, NST, 128], F16)
    vsb_all = res.tile([64, B, NST, 128], F16)
    vaug_all = res.tile([128, NST, B, 65], BF16)
    att_all = res.tile([128, NT, 128], F16)
    wo_sb = res.tile([128, 1024], F16)
    c1_sb = res.tile([128, NST, 12], F32)
    c2it_sb = res.tile([1, 48], F32)
    itau_sb = res.tile([128, 3], F32)
    maskt_sb = res.tile([128, 128], F32)
    ident_sb = res.tile([128, 128], F16)
    ones_sb = res.tile([128, 1], F32)
    geps_sb = res.tile([128, 1], F32)
    gvqk_sb = res.tile([128, B], F32)
    gvv_sb = res.tile([128, B], F32)
    gates_sb = res.tile([1, 48], F16)
    g48_sb = res.tile([48, 1], F16)
    g48x4_sb = res.tile([48, B], F16)
    selqk_sb = res.tile([48, 128], F16)
    selv_sb = res.tile([48, 128], F16)
    bmask_sb = res.tile([48, B], F16)

    # resident loads (reordered APs from host-contiguous arrays)
    wq = g['wqkv16'][:]
    dma(out=wqkv_sb[:], in_=bass.AP(tensor=wq.tensor, offset=wq.offset,
        ap=[[128, 128], [4 * 128 * 128, 8], [128 * 128, 4], [1, 128]]))
    ww = g['wws'][:]
    dma(out=wws_sb[:], in_=bass.AP(tensor=ww.tensor, offset=ww.offset,
        ap=[[128, 128], [128 * 128, 5], [1, 128]]))
    wp = g['wpb'][:]
    dma(out=wpb_sb[:], in_=bass.AP(tensor=wp.tensor, offset=wp.offset,
        ap=[[128, 128], [128 * 128, 10], [1, 128]]))
    ss = g['selsum'][:]
    dma(out=selsum_sb[:], in_=bass.AP(tensor=ss.tensor, offset=ss.offset,
        ap=[[18, 128], [128 * 18, 24], [1, 18]]))
    rt = g['ropetabs'][:]
    dma(out=rope_sb[:], in_=bass.AP(tensor=rt.tensor, offset=rt.offset,
        ap=[[128, 128], [NST * 128 * 128, 4], [128 * 128, NST], [1, 128]]))
    dma(out=wo_sb[:], in_=g['wo16'][:])
    dma(out=c1_sb[:], in_=g['c1_12'][:])
    c2 = g['c2it48'][:]
    dma(out=c2it_sb[:], in_=bass.AP(tensor=c2.tensor, offset=c2.offset,
                                    ap=[[0, 1], [1, 48]]))
    it = g['itau3'][:]
    dma(out=itau_sb[:], in_=bass.AP(tensor=it.tensor, offset=it.offset,
                                    ap=[[0, 128], [1, 3]]))
    dma(out=maskt_sb[:], in_=g['maskt'][:])
    dma(out=selqk_sb[:], in_=g['selqk'][:])
    dma(out=selv_sb[:], in_=g['selv48'][:])
    dma(out=bmask_sb[:], in_=g['bmask'][:])
    make_identity(nc, ident_sb[:])
    nc.vector.memset(ones_sb[:], 1.0)
    nc.vector.memset(geps_sb[:], GEPS)
    nc.vector.memset(vaug_all[:, :, :, 64:65], 1.0)

    # ---------------- phase A ----------------
    if PHASES < 1:
        return
    with tc.tile_pool(name="pa" + sfx, bufs=2) as pa, \
         tc.tile_pool(name="pga" + sfx, bufs=1, space="PSUM") as pga, \
         tc.tile_pool(name="pgb" + sfx, bufs=1, space="PSUM") as pgb:
        for st in range(NST):
            wcat_t = pa.tile([128, 4, 4, 128], F16, tag="wcat")
            wc = g['wcatT'][:][st]
            dma(out=wcat_t[:], in_=bass.AP(tensor=wc.tensor, offset=wc.offset,
                ap=[[128, 128], [4 * 128 * 128, 4], [128 * 128, 4], [1, 128]]))
            xt = pa.tile([128, B, 8, 128], F16, tag="xt")
            dma(out=xt[:], in_=g['xt16'][:][st])
            bc_st = bcat_all[:, st, :, :, :]          # [128, 4, B, 128]
            # GEMM: 4 M-chunks x 8 K-chunks, N=512 (all b)
            for mc in range(4):
                gem = pga.tile([128, 512], F32, tag=f"gem{mc % 2}")
                for k in range(8):
                    rhs = _ap(xt[:, 0, k, :], 0, [[8 * 128, B], [1, 128]])
                    nc.tensor.matmul(gem[:], wqkv_sb[:, k, mc, :], rhs,
                                     start=(k == 0), stop=(k == 7))
                if mc % 2 == 0:
                    nc.vector.tensor_copy(bc_st[:, mc, :, :], gem[:])
                else:
                    nc.scalar.copy(bc_st[:, mc, :, :], gem[:])
            if PHASES < 2:
                continue
            sa0 = stats_all[:, st, 0, 0:1]
            # w = bilinear helper rows (f32 psum)
            wt = pgb.tile([128, 4, 512], F32, tag="ps4")
            nc.tensor.matmul(wt[:, 0, :], wws_sb[:, 0, :], bc_st[:, 0, :, :],
                             start=True, stop=False)
            nc.tensor.matmul(wt[:, 0, :], wws_sb[:, 1, :], bc_st[:, 1, :, :],
                             start=False, stop=True)
            nc.tensor.matmul(wt[:, 1, :], wws_sb[:, 2, :], bc_st[:, 1, :, :])
            nc.tensor.matmul(wt[:, 2, :], wws_sb[:, 3, :], bc_st[:, 2, :, :])
            nc.tensor.matmul(wt[:, 3, :], wws_sb[:, 4, :], bc_st[:, 3, :, :])
            wsb = pa.tile([128, 4, 512], F16, tag="wsb")
            nc.vector.tensor_copy(wsb[:, 0:2, :], wt[:, 0:2, :])
            nc.scalar.copy(wsb[:, 2:4, :], wt[:, 2:4, :])
            if PHASES < 3:
                continue
            # z products: z_e' = bcat (.) wcatT_e' (b-broadcast weights)
            zs = [pa.tile([128, 4, 512], F16, tag=f"z{e}", name=f"z{e}")
                  for e in range(4)]
            for e in range(4):
                w0 = wcat_t[:, e, 0, 0:1]
                win = _ap(w0, 0, [[128, 4], [0, B], [1, 128]])
                if e < 3:
                    nc.vector.tensor_mul(zs[e][:], bc_st[:], win)
                else:
                    nc.gpsimd.tensor_mul(zs[e][:], bc_st[:], win)
            z2t = pa.tile([128, 4, 512], F16, tag="z2t")
            nc.vector.tensor_mul(z2t[:], bc_st[:], wsb[:])
            # partition-sums into P [18, (b,tok)]
            Pps = pgb.tile([18, 512], F32, tag="Pps")
            nmm = 24
            i = 0
            for e in range(4):
                for ci in range(4):
                    nc.tensor.matmul(Pps[:], selsum_sb[:, e * 4 + ci, :],
                                     zs[e][:, ci, :], start=(i == 0),
                                     stop=(i == nmm - 1))
                    i += 1
            for ci in range(4):
                nc.tensor.matmul(Pps[:], selsum_sb[:, 16 + ci, :],
                                 z2t[:, ci, :], start=False, stop=False)
                i += 1
                nc.tensor.matmul(Pps[:], selsum_sb[:, 20 + ci, :],
                                 bc_st[:, ci, :, :], start=False,
                                 stop=(i == nmm - 1))
                i += 1
            Psb = pa.tile([18, 512], F16, tag="Psb")
            nc.vector.tensor_copy(Psb[:], Pps[:])
            fixT = pgb.tile([128, B, 20], F16, tag="fixT")
            for b in range(B):
                nc.tensor.transpose(fixT[:, b, 0:18],
                                    Psb[:, b * 128:(b + 1) * 128],
                                    ident_sb[0:18, 0:18])
            fx = fixT[:, 0, 0:1]
            nc.vector.tensor_copy(_ap(sa0, 0, [[18, B], [1, 4], [5, 3]]),
                                  _ap(fx, 0, [[20, B], [3, 4], [1, 3]]))
            nc.vector.tensor_copy(_ap(sa0, 15, [[18, B], [1, 3]]),
                                  _ap(fx, 12, [[20, B], [1, 3]]))
            nc.vector.tensor_copy(_ap(sa0, 4, [[18, B], [5, 3]]),
                                  _ap(fx, 15, [[20, B], [1, 3]]))

    if PHASES < 4:
        return
    # ---------------- AllReduce ----------------
    with tc.tile_pool(name="dr" + sfx, bufs=1, space="DRAM") as dr:
        ar_in = dr.tile([128, NST, B, 18], F16)
        ar_out = dr.tile([128, NST, B, 18], F16)
        dma(out=ar_in[:], in_=stats_all[:])
        if NOCOLL:
            dma(out=ar_out[:], in_=ar_in[:])
        else:
            nc.gpsimd.collective_compute(
                "AllReduce", OP.add, replica_groups=[list(range(NC))],
                ins=[ar_in[:]], outs=[ar_out[:]])
        dma(out=stats_ar[:], in_=ar_out[:])

    if PHASES < 5:
        return
    # ---------------- gates (replicated) ----------------
    with tc.tile_pool(name="pg" + sfx, bufs=1) as pg, \
         tc.tile_pool(name="pgp" + sfx, bufs=1, space="PSUM") as pgp:
        lpart = pg.tile([128, 48], F32)
        for pi, Ff in enumerate([NE * H * HD, NE * KVH * HD, NE * KVH * HD]):
            s1 = stats_ar[:, :, :, 5 * pi + 4]
            s2 = stats_ar[:, :, :, 15 + pi]
            mu = pg.tile([128, NST, B], F32, tag=f"mu{pi}")
            vr = pg.tile([128, NST, B], F32, tag=f"vr{pi}")
            rr = pg.tile([128, NST, B], F32, tag=f"rr{pi}")
            rm = pg.tile([128, NST, B], F32, tag=f"rm{pi}")
            t1 = pg.tile([128, NST, B, 4], F32, tag=f"t1{pi}")
            t2 = pg.tile([128, NST, B, 4], F32, tag=f"t2{pi}")
            nc.vector.tensor_scalar_mul(mu[:], s1, 1.0 / Ff)
            nc.vector.tensor_scalar_mul(vr[:], s2, 1.0 / Ff)
            nc.vector.tensor_mul(t2[:, :, :, 0], mu[:], mu[:])
            nc.vector.tensor_sub(vr[:], vr[:], t2[:, :, :, 0])
            nc.scalar.activation(out=rr[:], in_=vr[:], func=AF.Sqrt,
                                 bias=geps_sb[:], scale=1.0)
            nc.vector.reciprocal(rr[:], rr[:])
            nc.vector.tensor_scalar_mul(rr[:], rr[:], itau_sb[:, pi:pi + 1])
            nc.vector.tensor_mul(rm[:], rr[:], mu[:])
            nc.vector.tensor_mul(t1[:], stats_ar[:, :, :, 5 * pi:5 * pi + 4],
                                 _bcast_ap(rr[:], [(3, 4)]))
            nc.vector.tensor_mul(t2[:], _bcast_ap(c1_sb[:, :, 4 * pi:4 * pi + 4],
                                                  [(2, B)]),
                                 _bcast_ap(rm[:], [(3, 4)]))
            nc.vector.tensor_sub(t1[:], t1[:], t2[:])
            src = bass.AP(tensor=t1.tensor, offset=t1[:].offset,
                          ap=[list(t1[:].ap[0]), [4, B], [1, 4], [4 * B, NST]])
            dst = bass.AP(tensor=lpart.tensor, offset=lpart[:].offset + 16 * pi,
                          ap=[list(lpart[:].ap[0]), [4, B], [1, 4]])
            nc.vector.tensor_reduce(dst, src, axis=AX.X, op=OP.add)
        pl48 = pgp.tile([1, 48], F32)
        nc.tensor.matmul(pl48[:], ones_sb[:], lpart[:])
        lg = pg.tile([1, 48], F32)
        nc.vector.tensor_add(lg[:], pl48[:], c2it_sb[:])
        mx = pg.tile([1, 12], F32)
        nc.vector.tensor_reduce(mx[:], lg[:].rearrange('o (g e) -> o g e', e=4),
                                axis=AX.X, op=OP.max)
        for e in range(4):
            nc.vector.tensor_sub(lg[:].rearrange('o (g e) -> o g e', e=4)[:, :, e],
                                 lg[:].rearrange('o (g e) -> o g e', e=4)[:, :, e],
                                 mx[:])
        nc.scalar.activation(out=lg[:], in_=lg[:], func=AF.Exp)
        sm = pg.tile([1, 12], F32)
        nc.vector.tensor_reduce(sm[:], lg[:].rearrange('o (g e) -> o g e', e=4),
                                axis=AX.X, op=OP.add)
        nc.vector.reciprocal(sm[:], sm[:])
        for e in range(4):
            nc.vector.tensor_mul(gates_sb[:].rearrange('o (g e) -> o g e', e=4)[:, :, e],
                                 lg[:].rearrange('o (g e) -> o g e', e=4)[:, :, e],
                                 sm[:])
        g48T = pgp.tile([48, 1], F16)
        nc.tensor.transpose(g48T[:], gates_sb[:], ident_sb[0:1, 0:1])
        nc.scalar.copy(g48_sb[:], g48T[:])
        nc.vector.tensor_mul(g48x4_sb[:], bmask_sb[:],
                             _bcast_ap(g48_sb[:, 0], [(1, B)]))
        gvp = pgp.tile([128, B], F32)
        gvvp = pgp.tile([128, B], F32)
        nc.tensor.matmul(gvp[:], selqk_sb[:], g48x4_sb[:])
        nc.tensor.matmul(gvvp[:], selv_sb[:], g48x4_sb[:])
        nc.scalar.copy(gvqk_sb[:], gvp[:])
        nc.scalar.copy(gvv_sb[:], gvvp[:])

    if PHASES < 6:
        return
    # ---------------- phase B ----------------
    with tc.tile_pool(name="pb" + sfx, bufs=2) as pb, \
         tc.tile_pool(name="pxf" + sfx, bufs=1, space="PSUM") as pxf, \
         tc.tile_pool(name="pvt" + sfx, bufs=1, space="PSUM") as pvt:
        for b in range(B):
            # gate-scaled h (junk rows are zeroed by lora lhsT later)
            hg1 = pb.tile([128, NST, 128], F16, tag="hg1")
            hg2 = pb.tile([128, NST, 128], F16, tag="hg2")
            hg3 = pb.tile([128, NST, 128], F16, tag="hg3")
            for hgt, ci, gvt in ((hg1, 1, gvqk_sb), (hg2, 2, gvqk_sb),
                                 (hg3, 3, gvv_sb)):
                src = _ap(bcat_all[:, 0, ci, b, :], 0,
                          [[4 * B * 128, NST], [1, 128]])
                gvc = bass.AP(tensor=gvt.tensor, offset=gvt[:].offset + b,
                              ap=[list(gvt[:].ap[0]), [0, NST], [0, 128]])
                nc.vector.tensor_mul(hgt[:], src, gvc)
            for half in range(2):
                st0 = half * 4
                xs = [pxf.tile([128, 512], F32, tag=f"xf{ci}", name=f"xf{ci}")
                      for ci in range(5)]
                mmdefs = [(0, 0, 0, None), (0, 1, None, hg1),
                          (1, 2, 0, None), (1, 3, None, hg1),
                          (2, 4, 2, None), (2, 5, None, hg2),
                          (3, 6, 2, None), (3, 7, None, hg2),
                          (4, 8, 3, None), (4, 9, None, hg3)]
                for xi, wi, ci, hgt in mmdefs:
                    if hgt is None:
                        rhs = _ap(bcat_all[:, st0, ci, b, :], 0,
                                  [[4 * B * 128, 4], [1, 128]])
                    else:
                        rhs = hgt[:, st0:st0 + 4, :]
                    nc.tensor.matmul(xs[xi][:], wpb_sb[:, wi, :], rhs,
                                     start=(wi % 2 == 0), stop=(wi % 2 == 1))
                # rope: xr = C(.)x + S(.)xp
                t1 = pb.tile([128, 512], F16, tag="t1")
                t2 = pb.tile([128, 512], F16, tag="t2")
                t3 = pb.tile([128, 512], F16, tag="t3")
                t4 = pb.tile([128, 512], F16, tag="t4")
                nc.vector.tensor_mul(t1[:], rope_sb[:, 0, st0:st0 + 4, :], xs[0][:])
                nc.vector.tensor_mul(t2[:], rope_sb[:, 1, st0:st0 + 4, :], xs[1][:])
                nc.vector.tensor_add(xrq_all[:, b, st0:st0 + 4, :], t1[:], t2[:])
                nc.vector.tensor_mul(t3[:], rope_sb[:, 2, st0:st0 + 4, :], xs[2][:])
                nc.vector.tensor_mul(t4[:], rope_sb[:, 3, st0:st0 + 4, :], xs[3][:])
                nc.gpsimd.tensor_add(xrkv_all[:, b, st0:st0 + 4, :], t3[:], t4[:])
                nc.scalar.copy(vsb_all[:, b, st0:st0 + 4, :], xs[4][0:64, :])
            # vaug transposes
            vtr = pvt.tile([128, NST, 64], F16, tag="vtr")
            for st in range(NST):
                nc.tensor.transpose(vtr[:, st, :], vsb_all[:, b, st, :],
                                    ident_sb[0:64, 0:64])
            nc.vector.tensor_copy(
                _ap(vaug_all[:, 0, b, 0:1], 0, [[B * 65, NST], [1, 64]]), vtr[:])

    if PHASES < 7:
        return
    # ---------------- attention ----------------
    with tc.tile_pool(name="pat" + sfx, bufs=2) as pat, \
         tc.tile_pool(name="pps" + sfx, bufs=2, space="PSUM") as pps, \
         tc.tile_pool(name="ppa" + sfx, bufs=2, space="PSUM") as ppa:
        for b in range(B):
            for hh in range(2):
                h0 = 64 * hh
                probs = pat.tile([128, NST, 1024], BF16, tag="probs")
                for i in range(NST):
                    ki = xrkv_all[h0:h0 + 64, b, i, :]
                    c0 = i * 128
                    ps = pps.tile([128, 1024], F32, tag="ps")
                    for (m0, m1) in [(c0, 512), (max(512, c0), 1024)]:
                        if m0 >= m1:
                            continue
                        rhs = xrq_all[h0:h0 + 64, b, m0 // 128:m1 // 128, :]
                        nc.tensor.matmul(ps[:, m0:m1], ki, rhs)
                    nc.vector.tensor_add(ps[:, c0:c0 + 128], ps[:, c0:c0 + 128],
                                         maskt_sb[:])
                    nc.scalar.activation(out=probs[:, i, c0:1024],
                                         in_=ps[:, c0:1024], func=AF.Exp)
                pa8 = ppa.tile([128, NST, 128], F32, tag="pa8")
                for j in range(NST):
                    for i in range(j + 1):
                        nc.tensor.matmul(pa8[:, j, 0:65],
                                         probs[:, i, 128 * j:128 * j + 128],
                                         vaug_all[:, i, b, :],
                                         start=(i == 0), stop=(i == j))
                rc8 = pat.tile([128, NST], F32, tag="rc8")
                pav = pa8[:]
                nc.vector.reciprocal(
                    rc8[:], bass.AP(tensor=pa8.tensor, offset=pav.offset + 64,
                                    ap=[list(pav.ap[0]), [128, NST]]))
                ad0 = att_all[:, b, h0:h0 + 64]
                nc.vector.tensor_mul(
                    bass.AP(tensor=att_all.tensor, offset=ad0.offset,
                            ap=[list(ad0.ap[0]), [B * 128, NST], [1, 64]]),
                    pa8[:, :, 0:64], _bcast_ap(rc8[:], [(2, 64)]))

    if PHASES < 8:
        return
    # ---------------- output projection ----------------
    with tc.tile_pool(name="pw" + sfx, bufs=2) as pw, \
         tc.tile_pool(name="ppo" + sfx, bufs=2, space="PSUM") as ppo, \
         tc.tile_pool(name="ppat" + sfx, bufs=2, space="PSUM") as ppat:
        for b in range(B):
            paT = ppat.tile([128, NST, 128], F16, tag="paT")
            for st in range(NST):
                nc.tensor.transpose(paT[:, st, :], att_all[:, st * B + b, :],
                                    ident_sb[:])
            aT = pw.tile([128, NST, 128], F16, tag="aT")
            nc.scalar.copy(aT[:], paT[:])
            ob = pw.tile([128, NST, 1024], F16, tag="ob")
            for st in range(NST):
                po = ppo.tile([128, 1024], F32, tag="po")
                nc.tensor.matmul(po[:, 0:512], aT[:, st, :], wo_sb[:, 0:512])
                nc.tensor.matmul(po[:, 512:1024], aT[:, st, :], wo_sb[:, 512:1024])
                if st % 2 == 0:
                    nc.scalar.copy(ob[:, st, :], po[:])
                else:
                    nc.vector.tensor_copy(ob[:, st, :], po[:])
            dma(out=g['outp'][:][b], in_=ob[:])


def build_kernel(repeat=1, loopn=0):
    key = (repeat, loopn, NOCOLL, PHASES)
    if key in _CACHE:
        return _CACHE[key]
    nc = bacc.Bacc()
    io = {}
    def din(name, shape, dt):
        io[name] = nc.dram_tensor(name, list(shape), dt, kind="ExternalInput")
    din('xt16', (NST, 128, B, 8, 128), F16)
    din('wqkv16', (8, 4, 128, 128), F16)
    din('wcatT', (NST, 4, 4, 128, 128), F16)
    din('wws', (5, 128, 128), F16)
    din('wpb', (10, 128, 128), F16)
    din('selsum', (24, 128, 18), F16)
    din('ropetabs', (4, NST, 128, 128), F16)
    din('c1_12', (128, NST, 12), F32)
    din('c2it48', (48,), F32)
    din('itau3', (3,), F32)
    din('maskt', (128, 128), F32)
    din('selqk', (48, 128), F16)
    din('selv48', (48, 128), F16)
    din('bmask', (48, B), F16)
    din('wo16', (128, 1024), F16)
    io['outp'] = nc.dram_tensor('outp', [B, 128, NST, 1024], F16,
                                kind="ExternalOutput")
    nc._kernel_io = io
    from contextlib import ExitStack
    with tile.TileContext(nc) as tc:
        if loopn:
            with tc.For_i(0, loopn):
                with ExitStack() as ctx:
                    _emit(nc, tc, ctx, 0)
        else:
            for rep in range(repeat):
                with ExitStack() as ctx:
                    _emit(nc, tc, ctx, rep)
    nc.finalize()
    _CACHE[key] = nc
    return nc


def prep_inputs(inputs):
    f = np.float32
    x = np.asarray(inputs['x'], f)
    xr8 = np.asarray(x.transpose(2, 0, 1), np.float16).reshape(8, 128, B, NST, 128)
    xt16 = np.ascontiguousarray(xr8.transpose(3, 1, 2, 0, 4))

    wq = np.asarray(inputs['wq'], f)
    wk = np.asarray(inputs['wk'], f)
    wv = np.asarray(inputs['wv'], f)
    wo = np.asarray(inputs['wo'], f)

    pr = {}
    for p, Of in [('q', H * HD), ('k', KVH * HD), ('v', KVH * HD)]:
        A = np.asarray(inputs[f'A_{p}'], f)
        Bm = np.asarray(inputs[f'B_{p}'], f)
        gg = np.asarray(inputs[f'g_{p}'], f)
        bb = np.asarray(inputs[f'b_{p}'], f)
        We = np.asarray(inputs[f'We_{p}'], f)
        tau = float(np.asarray(inputs[f'tau_{p}']))
        itau = 1.0 / max(tau, 1e-6)
        Acat = np.ascontiguousarray(A.transpose(1, 0, 2).reshape(D, NE * R))
        Bp = SCALING * Bm
        Bflat = Bp.reshape(NE * R, Of)
        gv = gg.reshape(NE, Of)
        Wgf = We.reshape(S, NE, Of, NE) * gv[None, :, :, None]
        Wgb = Wgf.sum(axis=1)
        W2 = np.einsum('ero,seoE->serE', Bp, Wgf).reshape(S, NE * R, NE) / NC
        b2 = Bp.sum(axis=2).reshape(NE * R) / NC
        C1 = Wgf.sum(axis=(1, 2)) * itau
        C2 = (We.reshape(S, NE * Of, NE) * bb[None, :, None]).sum((0, 1)) * itau
        pr[p] = dict(Acat=Acat, Bp=Bp, Bflat=Bflat, Wgb=Wgb, W2=W2, b2=b2,
                     C1=C1, C2=C2, itau=itau)

    c1_12 = np.ascontiguousarray(
        np.concatenate([pr[p]['C1'] for p in 'qkv'], 1).astype(f)
        .reshape(NST, 128, 12).transpose(1, 0, 2))
    c2it48 = np.zeros(48, f)
    for pi, p in enumerate('qkv'):
        for b in range(B):
            c2it48[pi * 16 + b * 4:pi * 16 + b * 4 + 4] = pr[p]['C2']
    itau3 = np.array([pr[p]['itau'] for p in 'qkv'], f)
    maskt = np.ascontiguousarray(np.asarray(inputs['mask'], f)[0:128, 0:128].T)

    cos = np.asarray(inputs['cos'], f)
    sin = np.asarray(inputs['sin'], f)
    Cfull = np.zeros((64, S), f)
    Sfull = np.zeros((64, S), f)
    for i in range(32):
        Cfull[2 * i] = cos[:, i]
        Cfull[2 * i + 1] = cos[:, i]
        Sfull[2 * i] = -sin[:, i]
        Sfull[2 * i + 1] = sin[:, i]
    CA = np.concatenate([Cfull, Cfull], 0) * 0.125
    SA = np.concatenate([Sfull, Sfull], 0) * 0.125
    CB = np.concatenate([Cfull, Cfull], 0)
    SB = np.concatenate([Sfull, Sfull], 0)
    ropetabs = np.stack([CA, SA, CB, SB], 0).reshape(4, 128, NST, 128)
    ropetabs = np.ascontiguousarray(ropetabs.transpose(0, 2, 1, 3)).astype(np.float16)

    selqk = np.zeros((48, 128), np.float16)
    for er in range(128):
        pi, e = er // 64, (er % 64) // 16
        for b in range(B):
            selqk[pi * 16 + b * 4 + e, er] = 1.0
    selv48 = np.zeros((48, 128), np.float16)
    for er in range(64):
        for b in range(B):
            selv48[2 * 16 + b * 4 + er // 16, 64 + er] = 1.0
    bmask = np.zeros((48, B), np.float16)
    for fl in range(48):
        bmask[fl, (fl % 16) // 4] = 1.0

    P = np.zeros((128, 128), f)
    for m in range(128):
        P[m ^ 1, m] = 1.0

    in_maps = []
    for c in range(NC):
        qs = slice(128 * c, 128 * c + 128)
        ks = slice(64 * c, 64 * c + 64)
        mcols = [wq[:, qs],
                 np.concatenate([pr['q']['Acat'],
                                 np.zeros((D, 64), f)], 1),
                 np.concatenate([wk[:, ks], pr['k']['Acat']], 1),
                 np.concatenate([wv[:, ks], pr['v']['Acat']], 1)]
        wqkv16 = np.stack([m.reshape(8, 128, 128) for m in mcols], 1)
        wqkv16 = np.ascontiguousarray(wqkv16.astype(np.float16))

        pi_c = [np.full(128, 0),
                np.concatenate([np.full(64, 0), np.full(64, -1)]),
                np.full(128, 1), np.full(128, 2)]
        selsum = np.zeros((24, 128, 18), np.float16)
        for e in range(4):
            for ci in range(4):
                for frow in range(128):
                    pi = pi_c[ci][frow]
                    if pi >= 0:
                        selsum[e * 4 + ci, frow, 3 * e + pi] = 1.0
        for ci in range(4):
            for frow in range(128):
                pi = pi_c[ci][frow]
                if pi >= 0:
                    selsum[16 + ci, frow, 12 + pi] = 1.0
        selsum[20, :, 15] = 4.0
        selsum[21, 0:64, 15] = pr['q']['b2'].astype(np.float16)
        selsum[22, 0:64, 16] = 4.0
        selsum[22, 64:128, 16] = pr['k']['b2'].astype(np.float16)
        selsum[23, 0:64, 17] = 4.0
        selsum[23, 64:128, 17] = pr['v']['b2'].astype(np.float16)

        wcatT = np.zeros((NST, 4, 4, 128, 128), np.float16)
        for e in range(4):
            w0 = pr['q']['Wgb'][:, qs, e].T
            w1 = np.zeros((128, S), f)
            w1[0:64] = pr['q']['W2'][:, :, e].T
            w2 = np.zeros((128, S), f)
            w2[0:64] = pr['k']['Wgb'][:, ks, e].T
            w2[64:128] = pr['k']['W2'][:, :, e].T
            w3 = np.zeros((128, S), f)
            w3[0:64] = pr['v']['Wgb'][:, ks, e].T
            w3[64:128] = pr['v']['W2'][:, :, e].T
            for ci, w in enumerate([w0, w1, w2, w3]):
                wcatT[:, e, ci] = w.reshape(128, NST, 128).transpose(1, 0, 2)

        WW0a = (4.0 * np.eye(128, dtype=f)).astype(np.float16)
        t = np.zeros((128, 128), f)
        t[0:64] = 2.0 * pr['q']['Bflat'][:, qs]
        WW0b = t.astype(np.float16)
        Gq = np.zeros((64, 64), f)
        Gk = np.zeros((64, 64), f)
        Gv = np.zeros((64, 64), f)
        for e in range(NE):
            bq = pr['q']['Bp'][e][:, qs]
            bk = pr['k']['Bp'][e][:, ks]
            bvv = pr['v']['Bp'][e][:, ks]
            Gq[e * 16:(e + 1) * 16, e * 16:(e + 1) * 16] = bq @ bq.T
            Gk[e * 16:(e + 1) * 16, e * 16:(e + 1) * 16] = bk @ bk.T
            Gv[e * 16:(e + 1) * 16, e * 16:(e + 1) * 16] = bvv @ bvv.T
        t = np.zeros((128, 128), f)
        t[0:64, 0:64] = Gq
        WW1 = t.astype(np.float16)
        t = np.zeros((128, 128), f)
        t[0:64, 0:64] = 4.0 * np.eye(64)
        t[64:128, 0:64] = 2.0 * pr['k']['Bflat'][:, ks]
        t[64:128, 64:128] = Gk
        WW2 = t.astype(np.float16)
        t = np.zeros((128, 128), f)
        t[0:64, 0:64] = 4.0 * np.eye(64)
        t[64:128, 0:64] = 2.0 * pr['v']['Bflat'][:, ks]
        t[64:128, 64:128] = Gv
        WW3 = t.astype(np.float16)
        wws = np.stack([WW0a, WW0b, WW1, WW2, WW3], 0)

        def pad(a, r0, c0):
            t = np.zeros((128, 128), f)
            t[r0:r0 + a.shape[0], c0:c0 + a.shape[1]] = a
            return t
        WA_base = 2.0 * np.eye(128, dtype=f)
        WA_lora = pad(pr['q']['Bflat'][:, qs], 0, 0)
        WB_base = pad(2.0 * np.eye(64), 0, 0) + pad(2.0 * np.eye(64), 0, 64)
        WB_lora = (pad(pr['k']['Bflat'][:, ks], 64, 0)
                   + pad(pr['k']['Bflat'][:, ks], 64, 64))
        WC_base = pad(2.0 * np.eye(64), 0, 0)
        WC_lora = pad(pr['v']['Bflat'][:, ks], 64, 0)
        wpb = np.stack([WA_base, WA_lora, WA_base @ P, WA_lora @ P,
                        WB_base, WB_lora, WB_base @ P, WB_lora @ P,
                        WC_base, WC_lora], 0).astype(np.float16)

        m = dict(xt16=xt16, wqkv16=wqkv16, wcatT=wcatT, wws=wws, wpb=wpb,
                 selsum=selsum, ropetabs=ropetabs, c1_12=c1_12, c2it48=c2it48,
                 itau3=itau3, maskt=maskt, selqk=selqk, selv48=selv48,
                 bmask=bmask, wo16=wo[qs, :].astype(np.float16))
        in_maps.append(m)
    return in_maps


def run_on_device(in_maps, repeat=1, loopn=0):
    from concourse.bass_utils import run_bass_kernel_spmd
    nc = build_kernel(repeat, loopn)
    res = run_bass_kernel_spmd(nc, in_maps, list(range(NC)))
    return res


def _run_sim(in_maps):
    from concourse.bass_interp import MultiCoreSim
    nc = build_kernel(1)
    sim = MultiCoreSim(nc, NC, num_workers=NC)
    for c in range(NC):
        for name, arr in in_maps[c].items():
            sim.cores[c].tensor(name)[:] = arr
    sim.simulate()
    return [{'outp': np.asarray(sim.cores[c].tensor('outp'))} for c in range(NC)]


def kernel(**inputs):
    in_maps = prep_inputs(inputs)
    try:
        results = run_on_device(in_maps, repeat=1).results
    except Exception as e:
        sys.stderr.write(f"device run failed ({e}); falling back to CoreSim\n")
        results = _run_sim(in_maps)
    out = np.zeros((B, 128, NST, 1024), np.float32)
    for c in range(NC):
        out += np.asarray(results[c]['outp'], np.float32)
    return np.ascontiguousarray(out.transpose(0, 2, 1, 3)).reshape(B, S, 1024)


# revision 4
# speedup vs baseline: 2.1472x; 1.4522x over previous
# Trainium2 Bass kernel v2 for nn_AttentionWithMoLE — 8-core TP over heads.
#
# v2 redesign: phase-A GEMM emitted TRANSPOSED ([feature, token] rows) so the
# MoLE stats ride the PE via gram/diagonal matmuls instead of Pool/DVE/Act
# elementwise sweeps:
#   bcat chunks (per st, all 4 b batched, N=512):
#     ch0 = base_q(128) ; ch1 = [h_q 64 | s1 3 | pad] ; ch2 = [base_k | h_k] ;
#     ch3 = [base_v | h_v]
#   P1[tok,e'] = diag of gram(wcatT_e', bcat)  (per-position weights as lhsT)
#   s2 = diag of gram(bcat_b, w_b) with w = [E*base+2u ; G h] built by 5 PE
#     matmuls from host-folded bilinear forms (u = Bflat^T h, G = blockdiag BB^T)
#   s1 = GEMM columns, tiny PE transposes into stats layout.
# Phase B builds xq/xk/xv transposed via PE (base/lora/permuted-rope weight
# folding), RoPE = C (.) x + S (.) (P x) on DVE with host tables; k rows are
# duplicated into both partition bands so both q heads find their k in-band.
# Attention/out-projection follow the v1 scheme (scoresT, exp, fused rowsum
# via ones-column, recip-normalize, wo with host-summed partials).
import sys
import numpy as np

sys.path.insert(0, '/opt/trn_rl_repo')

import concourse.bass as bass
import concourse.bacc as bacc
import concourse.tile as tile
import concourse.mybir as mybir
from concourse.masks import make_identity

NC = 8
B, S, D, H, KVH, HD, NE, R = 4, 1024, 1024, 16, 8, 64, 4, 16
SCALING, GEPS = 2.0, 1e-6
NST = S // 128           # 8 s-tiles
NT = NST * B             # 32 token tiles of 128
F16 = mybir.dt.float16
BF16 = mybir.dt.bfloat16
F32 = mybir.dt.float32
AX = mybir.AxisListType
OP = mybir.AluOpType
AF = mybir.ActivationFunctionType

_CACHE = {}
NOCOLL = False   # timing mode: collectives not supported inside For_i loops
PHASES = 9


def ml_bf16():
    import ml_dtypes
    return ml_dtypes.bfloat16


def _ap(base, doff, dims):
    """AP with base's partition dim, extra element offset doff, free dims."""
    return bass.AP(tensor=base.tensor, offset=base.offset + doff,
                   ap=[list(base.ap[0])] + [list(d) for d in dims])


def _bcast_ap(ap, ins):
    dims = [list(d) for d in ap.ap]
    for pos, count in ins:
        dims.insert(pos, [0, count])
    return bass.AP(tensor=ap.tensor, offset=ap.offset, ap=dims)


def _emit(nc, tc, ctx, rep):
    g = nc._kernel_io
    sfx = f"_r{rep}"
    dma = nc.sync.dma_start

    # ---------------- resident pool ----------------
    res = ctx.enter_context(tc.tile_pool(name="res" + sfx, bufs=1))
    wqkv_sb = res.tile([128, 8, 4, 128], F16)
    wws_sb = res.tile([128, 5, 128], F16)
    wpb_sb = res.tile([128, 10, 128], F16)
    selsum_sb = res.tile([128, 24, 18], F16)
    rope_sb = res.tile([128, 4, NST, 128], F16)
    bcat_all = res.tile([128, NST, 4, B, 128], F16)
    stats_all = res.tile([128, NST, B, 18], F16)
    stats_ar = res.tile([128, NST, B, 18], F16)
    xrq_all = res.tile([128, B, NST, 128], F16)
    xrkv_all = res.tile([128, B, NST, 128], F16)
    vsb_all = res.tile([64, B, NST, 128], F16)
    vaug_all = res.tile([128, NST, B, 65], BF16)
    att_all = res.tile([128, NT, 128], F16)
    wo_sb = res.tile([128, 1024], F16)
    c1_sb = res.tile([128, NST, 12], F32)
    c2it_sb = res.tile([1, 48], F32)
    itau_sb = res.tile([128, 3], F32)
    maskm_sb = res.tile([128, 128], BF16)
    ident_sb = res.tile([128, 128], F16)
    ones_sb = res.tile([128, 1], F32)
    geps_sb = res.tile([128, 1], F32)
    gvqk_sb = res.tile([128, B], F32)
    gvv_sb = res.tile([128, B], F32)
    gates_sb = res.tile([1, 48], F16)
    g48_sb = res.tile([48, 1], F16)
    g48x4_sb = res.tile([48, B], F16)
    selqk_sb = res.tile([48, 128], F16)
    selv_sb = res.tile([48, 128], F16)
    bmask_sb = res.tile([48, B], F16)

    # resident loads (reordered APs from host-contiguous arrays)
    wq = g['wqkv16'][:]
    dma(out=wqkv_sb[:], in_=bass.AP(tensor=wq.tensor, offset=wq.offset,
        ap=[[128, 128], [4 * 128 * 128, 8], [128 * 128, 4], [1, 128]]))
    ww = g['wws'][:]
    dma(out=wws_sb[:], in_=bass.AP(tensor=ww.tensor, offset=ww.offset,
        ap=[[128, 128], [128 * 128, 5], [1, 128]]))
    wp = g['wpb'][:]
    dma(out=wpb_sb[:], in_=bass.AP(tensor=wp.tensor, offset=wp.offset,
        ap=[[128, 128], [128 * 128, 10], [1, 128]]))
    ss = g['selsum'][:]
    dma(out=selsum_sb[:], in_=bass.AP(tensor=ss.tensor, offset=ss.offset,
        ap=[[18, 128], [128 * 18, 24], [1, 18]]))
    rt = g['ropetabs'][:]
    dma(out=rope_sb[:], in_=bass.AP(tensor=rt.tensor, offset=rt.offset,
        ap=[[128, 128], [NST * 128 * 128, 4], [128 * 128, NST], [1, 128]]))
    dma(out=wo_sb[:], in_=g['wo16'][:])
    dma(out=c1_sb[:], in_=g['c1_12'][:])
    c2 = g['c2it48'][:]
    dma(out=c2it_sb[:], in_=bass.AP(tensor=c2.tensor, offset=c2.offset,
                                    ap=[[0, 1], [1, 48]]))
    it = g['itau3'][:]
    dma(out=itau_sb[:], in_=bass.AP(tensor=it.tensor, offset=it.offset,
                                    ap=[[0, 128], [1, 3]]))
    dma(out=maskm_sb[:], in_=g['maskm'][:])
    dma(out=selqk_sb[:], in_=g['selqk'][:])
    dma(out=selv_sb[:], in_=g['selv48'][:])
    dma(out=bmask_sb[:], in_=g['bmask'][:])
    make_identity(nc, ident_sb[:])
    nc.vector.memset(ones_sb[:], 1.0)
    nc.vector.memset(geps_sb[:], GEPS)
    nc.vector.memset(vaug_all[:, :, :, 64:65], 1.0)

    # ---------------- phase A ----------------
    if PHASES < 1:
        return
    with tc.tile_pool(name="pa" + sfx, bufs=2) as pa, \
         tc.tile_pool(name="pga" + sfx, bufs=1, space="PSUM") as pga, \
         tc.tile_pool(name="pgb" + sfx, bufs=1, space="PSUM") as pgb:
        for st in range(NST):
            wcat_t = pa.tile([128, 4, 4, 128], F16, tag="wcat")
            wc = g['wcatT'][:][st]
            dma(out=wcat_t[:], in_=bass.AP(tensor=wc.tensor, offset=wc.offset,
                ap=[[128, 128], [4 * 128 * 128, 4], [128 * 128, 4], [1, 128]]))
            xt = pa.tile([128, B, 8, 128], F16, tag="xt")
            dma(out=xt[:], in_=g['xt16'][:][st])
            bc_st = bcat_all[:, st, :, :, :]          # [128, 4, B, 128]
            # GEMM: 4 M-chunks x 8 K-chunks, N=512 (all b)
            for mc in range(4):
                gem = pga.tile([128, 512], F32, tag=f"gem{mc % 2}")
                for k in range(8):
                    rhs = _ap(xt[:, 0, k, :], 0, [[8 * 128, B], [1, 128]])
                    nc.tensor.matmul(gem[:], wqkv_sb[:, k, mc, :], rhs,
                                     start=(k == 0), stop=(k == 7))
                if mc == 0:
                    nc.vector.tensor_copy(bc_st[:, mc, :, :], gem[:])
                else:
                    nc.scalar.copy(bc_st[:, mc, :, :], gem[:])
            if PHASES < 2:
                continue
            sa0 = stats_all[:, st, 0, 0:1]
            # w = bilinear helper rows (f32 psum)
            wt = pgb.tile([128, 4, 512], F32, tag="ps4")
            nc.tensor.matmul(wt[:, 0, :], wws_sb[:, 0, :], bc_st[:, 0, :, :],
                             start=True, stop=False)
            nc.tensor.matmul(wt[:, 0, :], wws_sb[:, 1, :], bc_st[:, 1, :, :],
                             start=False, stop=True)
            nc.tensor.matmul(wt[:, 1, :], wws_sb[:, 2, :], bc_st[:, 1, :, :])
            nc.tensor.matmul(wt[:, 2, :], wws_sb[:, 3, :], bc_st[:, 2, :, :])
            nc.tensor.matmul(wt[:, 3, :], wws_sb[:, 4, :], bc_st[:, 3, :, :])
            wsb = pa.tile([128, 4, 512], F16, tag="wsb")
            nc.vector.tensor_copy(wsb[:, 0:2, :], wt[:, 0:2, :])
            nc.scalar.copy(wsb[:, 2:4, :], wt[:, 2:4, :])
            if PHASES < 3:
                continue
            # z products: z_e' = bcat (.) wcatT_e' (b-broadcast weights)
            zs = [pa.tile([128, 4, 512], F16, tag=f"z{e}", name=f"z{e}")
                  for e in range(4)]
            for e in (3, 0, 1, 2):       # Pool op first so it overlaps DVE
                w0 = wcat_t[:, e, 0, 0:1]
                win = _ap(w0, 0, [[128, 4], [0, B], [1, 128]])
                if e == 3:
                    nc.gpsimd.tensor_mul(zs[e][:], bc_st[:], win)
                else:
                    nc.vector.tensor_mul(zs[e][:], bc_st[:], win)
            z2t = pa.tile([128, 4, 512], F16, tag="z2t")
            nc.vector.tensor_mul(z2t[:], bc_st[:], wsb[:])
            # partition-sums into P [18, (b,tok)]
            Pps = pgb.tile([18, 512], F32, tag="Pps")
            nmm = 24
            i = 0
            for e in range(4):
                for ci in range(4):
                    nc.tensor.matmul(Pps[:], selsum_sb[:, e * 4 + ci, :],
                                     zs[e][:, ci, :], start=(i == 0),
                                     stop=(i == nmm - 1))
                    i += 1
            for ci in range(4):
                nc.tensor.matmul(Pps[:], selsum_sb[:, 16 + ci, :],
                                 z2t[:, ci, :], start=False, stop=False)
                i += 1
                nc.tensor.matmul(Pps[:], selsum_sb[:, 20 + ci, :],
                                 bc_st[:, ci, :, :], start=False,
                                 stop=(i == nmm - 1))
                i += 1
            Psb = pa.tile([18, 512], F16, tag="Psb")
            nc.scalar.copy(Psb[:], Pps[:])
            fixT = pgb.tile([128, B, 20], F16, tag="fixT")
            for b in range(B):
                nc.tensor.transpose(fixT[:, b, 0:18],
                                    Psb[:, b * 128:(b + 1) * 128],
                                    ident_sb[0:18, 0:18])
            fx = fixT[:, 0, 0:1]
            nc.vector.tensor_copy(_ap(sa0, 0, [[18, B], [1, 4], [5, 3]]),
                                  _ap(fx, 0, [[20, B], [3, 4], [1, 3]]))
            nc.vector.tensor_copy(_ap(sa0, 15, [[18, B], [1, 3]]),
                                  _ap(fx, 12, [[20, B], [1, 3]]))
            nc.vector.tensor_copy(_ap(sa0, 4, [[18, B], [5, 3]]),
                                  _ap(fx, 15, [[20, B], [1, 3]]))

    if PHASES < 4:
        return
    # ---------------- AllReduce ----------------
    with tc.tile_pool(name="dr" + sfx, bufs=1, space="DRAM") as dr:
        ar_in = dr.tile([128, NST, B, 18], F16)
        ar_out = dr.tile([128, NST, B, 18], F16)
        dma(out=ar_in[:], in_=stats_all[:])
        if NOCOLL:
            dma(out=ar_out[:], in_=ar_in[:])
        else:
            nc.gpsimd.collective_compute(
                "AllReduce", OP.add, replica_groups=[list(range(NC))],
                ins=[ar_in[:]], outs=[ar_out[:]])
        dma(out=stats_ar[:], in_=ar_out[:])

    if PHASES < 5:
        return
    # ---------------- gates (replicated) ----------------
    with tc.tile_pool(name="pg" + sfx, bufs=1) as pg, \
         tc.tile_pool(name="pgp" + sfx, bufs=1, space="PSUM") as pgp:
        lpart = pg.tile([128, 48], F32)
        # batched over the 3 projections (mu/vr0 arrive pre-divided by Ff)
        sa_b = stats_ar[:, 0, 0, 0:1]
        mu3 = _ap(sa_b, 4, [[72, NST], [18, B], [5, 3]])
        vr03 = _ap(sa_b, 15, [[72, NST], [18, B], [1, 3]])
        vr = pg.tile([128, NST, B, 3], F32, tag="vr")
        rr = pg.tile([128, NST, B, 3], F32, tag="rr")
        rm = pg.tile([128, NST, B, 3], F32, tag="rm")
        nc.vector.tensor_mul(vr[:], mu3, mu3)
        nc.vector.tensor_sub(vr[:], vr03, vr[:])
        nc.scalar.activation(out=rr[:], in_=vr[:], func=AF.Sqrt,
                             bias=geps_sb[:], scale=1.0)
        nc.vector.reciprocal(rr[:], rr[:])
        for pi in range(3):
            nc.vector.tensor_scalar_mul(rr[:, :, :, pi], rr[:, :, :, pi],
                                        itau_sb[:, pi:pi + 1])
        nc.vector.tensor_mul(rm[:], rr[:], mu3)
        for pi in range(3):
            t1 = pg.tile([128, NST, B, 4], F32, tag=f"t1{pi}",
                         name=f"t1{pi}")
            t2 = pg.tile([128, NST, B, 4], F32, tag=f"t2{pi}",
                         name=f"t2{pi}")
            nc.vector.tensor_mul(t1[:], stats_ar[:, :, :, 5 * pi:5 * pi + 4],
                                 _bcast_ap(rr[:, :, :, pi], [(3, 4)]))
            nc.vector.tensor_mul(t2[:], _bcast_ap(c1_sb[:, :, 4 * pi:4 * pi + 4],
                                                  [(2, B)]),
                                 _bcast_ap(rm[:, :, :, pi], [(3, 4)]))
            nc.vector.tensor_sub(t1[:], t1[:], t2[:])
            src = bass.AP(tensor=t1.tensor, offset=t1[:].offset,
                          ap=[list(t1[:].ap[0]), [4, B], [1, 4], [4 * B, NST]])
            dst = bass.AP(tensor=lpart.tensor, offset=lpart[:].offset + 16 * pi,
                          ap=[list(lpart[:].ap[0]), [4, B], [1, 4]])
            nc.vector.tensor_reduce(dst, src, axis=AX.X, op=OP.add)
        pl48 = pgp.tile([1, 48], F32)
        nc.tensor.matmul(pl48[:], ones_sb[:], lpart[:])
        lg = pg.tile([1, 48], F32)
        nc.vector.tensor_add(lg[:], pl48[:], c2it_sb[:])
        mx = pg.tile([1, 12], F32)
        nc.vector.tensor_reduce(mx[:], lg[:].rearrange('o (g e) -> o g e', e=4),
                                axis=AX.X, op=OP.max)
        for e in range(4):
            nc.vector.tensor_sub(lg[:].rearrange('o (g e) -> o g e', e=4)[:, :, e],
                                 lg[:].rearrange('o (g e) -> o g e', e=4)[:, :, e],
                                 mx[:])
        nc.scalar.activation(out=lg[:], in_=lg[:], func=AF.Exp)
        sm = pg.tile([1, 12], F32)
        nc.vector.tensor_reduce(sm[:], lg[:].rearrange('o (g e) -> o g e', e=4),
                                axis=AX.X, op=OP.add)
        nc.vector.reciprocal(sm[:], sm[:])
        for e in range(4):
            nc.vector.tensor_mul(gates_sb[:].rearrange('o (g e) -> o g e', e=4)[:, :, e],
                                 lg[:].rearrange('o (g e) -> o g e', e=4)[:, :, e],
                                 sm[:])
        g48T = pgp.tile([48, 1], F16)
        nc.tensor.transpose(g48T[:], gates_sb[:], ident_sb[0:1, 0:1])
        nc.scalar.copy(g48_sb[:], g48T[:])
        nc.vector.tensor_mul(g48x4_sb[:], bmask_sb[:],
                             _bcast_ap(g48_sb[:, 0], [(1, B)]))
        gvp = pgp.tile([128, B], F32)
        gvvp = pgp.tile([128, B], F32)
        nc.tensor.matmul(gvp[:], selqk_sb[:], g48x4_sb[:])
        nc.tensor.matmul(gvvp[:], selv_sb[:], g48x4_sb[:])
        nc.scalar.copy(gvqk_sb[:], gvp[:])
        nc.scalar.copy(gvv_sb[:], gvvp[:])

    if PHASES < 6:
        return
    # -------- merged phase B + attention + output projection (per b) -------
    # PSUM tags: s0/s1 [128,512]F32 (xf waves + pa8 halves), big [128,1024]F32
    # (scores ps + po), tp16 [128,NST,128]F16 (vaug transposes + paT).
    with tc.tile_pool(name="pb" + sfx, bufs=2) as pb, \
         tc.tile_pool(name="pm" + sfx, bufs=1, space="PSUM") as pm, \
         tc.tile_pool(name="pbig" + sfx, bufs=2, space="PSUM") as pbig, \
         tc.tile_pool(name="ptp" + sfx, bufs=1, space="PSUM") as ptp:
        for b in range(B):
            # gate-scaled h (junk rows are zeroed by lora lhsT later)
            hg1 = pb.tile([128, NST, 128], F16, tag="hg1")
            hg2 = pb.tile([128, NST, 128], F16, tag="hg2")
            hg3 = pb.tile([128, NST, 128], F16, tag="hg3")
            for hgt, ci, gvt in ((hg1, 1, gvqk_sb), (hg2, 2, gvqk_sb),
                                 (hg3, 3, gvv_sb)):
                src = _ap(bcat_all[:, 0, ci, b, :], 0,
                          [[4 * B * 128, NST], [1, 128]])
                gvc = bass.AP(tensor=gvt.tensor, offset=gvt[:].offset + b,
                              ap=[list(gvt[:].ap[0]), [0, NST], [0, 128]])
                nc.vector.tensor_mul(hgt[:], src, gvc)
            for half in range(2):
                st0 = half * 4
                # waves on 2 rotating psum tags; base mms (gate-independent)
                # lead each accumulation. Rope reads of a wave are emitted
                # before the next wave reuses its banks.
                def wave(wi, ci, hgt, tagn, nm):
                    xt_ = pm.tile([128, 512], F32, tag=tagn, name=nm)
                    rhs = _ap(bcat_all[:, st0, ci, b, :], 0,
                              [[4 * B * 128, 4], [1, 128]])
                    nc.tensor.matmul(xt_[:], wpb_sb[:, wi, :], rhs,
                                     start=True, stop=False)
                    nc.tensor.matmul(xt_[:], wpb_sb[:, wi + 1, :],
                                     hgt[:, st0:st0 + 4, :],
                                     start=False, stop=True)
                    return xt_
                t1 = pb.tile([128, 512], F16, tag="t1")
                t2 = pb.tile([128, 512], F16, tag="t2")
                t3 = pb.tile([128, 512], F16, tag="t3")
                t4 = pb.tile([128, 512], F16, tag="t4")
                xA = wave(0, 0, hg1, "s0", "xA")
                xAp = wave(2, 0, hg1, "s1", "xAp")
                nc.vector.tensor_mul(t1[:], rope_sb[:, 0, st0:st0 + 4, :], xA[:])
                nc.vector.tensor_mul(t2[:], rope_sb[:, 1, st0:st0 + 4, :], xAp[:])
                nc.vector.tensor_add(xrq_all[:, b, st0:st0 + 4, :], t1[:], t2[:])
                xB = wave(4, 2, hg2, "s0", "xB")
                xBp = wave(6, 2, hg2, "s1", "xBp")
                nc.vector.tensor_mul(t3[:], rope_sb[:, 2, st0:st0 + 4, :], xB[:])
                nc.vector.tensor_mul(t4[:], rope_sb[:, 3, st0:st0 + 4, :], xBp[:])
                nc.gpsimd.tensor_add(xrkv_all[:, b, st0:st0 + 4, :], t3[:], t4[:])
                xC = wave(8, 3, hg3, "s0", "xC")
                nc.scalar.copy(vsb_all[:, b, st0:st0 + 4, :], xC[0:64, :])
            # vaug transposes
            vtr = ptp.tile([128, NST, 128], F16, tag="tp16", name="vtr")
            for st in range(NST):
                nc.tensor.transpose(vtr[:, st, 0:64], vsb_all[:, b, st, :],
                                    ident_sb[0:64, 0:64])
            nc.vector.tensor_copy(
                _ap(vaug_all[:, 0, b, 0:1], 0, [[B * 65, NST], [1, 64]]),
                vtr[:, :, 0:64])
            # ---- attention ----
            for hh in range(2):
                h0 = 64 * hh
                probs = pb.tile([128, NST, 1024], BF16, tag="probs")
                for i in range(NST):
                    ki = xrkv_all[h0:h0 + 64, b, i, :]
                    c0 = i * 128
                    ps = pbig.tile([128, 1024], F32, tag="big", name="ps")
                    for (m0, m1) in [(c0, 512), (max(512, c0), 1024)]:
                        if m0 >= m1:
                            continue
                        rhs = xrq_all[h0:h0 + 64, b, m0 // 128:m1 // 128, :]
                        nc.tensor.matmul(ps[:, m0:m1], ki, rhs)
                    nc.scalar.activation(out=probs[:, i, c0:1024],
                                         in_=ps[:, c0:1024], func=AF.Exp)
                    nc.gpsimd.tensor_mul(probs[:, i, c0:c0 + 128],
                                         probs[:, i, c0:c0 + 128], maskm_sb[:])
                for jh in range(2):
                    pa8 = pm.tile([128, 512], F32, tag=f"s{jh}",
                                  name=f"pa8{jh}")
                    pav = pa8[:].rearrange('p (j n) -> p j n', j=4)
                    for j4 in range(4):
                        j = jh * 4 + j4
                        for i in range(j + 1):
                            nc.tensor.matmul(pav[:, j4, 0:65],
                                             probs[:, i, 128 * j:128 * j + 128],
                                             vaug_all[:, i, b, :],
                                             start=(i == 0), stop=(i == j))
                    rc8 = pb.tile([128, 4], F32, tag="rc8")
                    nc.vector.reciprocal(
                        rc8[:], bass.AP(tensor=pa8.tensor,
                                        offset=pa8[:].offset + 64,
                                        ap=[list(pa8[:].ap[0]), [128, 4]]))
                    ad0 = att_all[:, jh * 4 * B + b, h0:h0 + 64]
                    nc.vector.tensor_mul(
                        bass.AP(tensor=att_all.tensor, offset=ad0.offset,
                                ap=[list(ad0.ap[0]), [B * 128, 4], [1, 64]]),
                        pav[:, :, 0:64], _bcast_ap(rc8[:], [(2, 64)]))
            # ---- output projection ----
            paT = ptp.tile([128, NST, 128], F16, tag="tp16", name="paT")
            for st in range(NST):
                nc.tensor.transpose(paT[:, st, :], att_all[:, st * B + b, :],
                                    ident_sb[:])
            aT = pb.tile([128, NST, 128], F16, tag="aT")
            nc.scalar.copy(aT[:], paT[:])
            ob = pb.tile([128, NST, 1024], F16, tag="ob")
            for st in range(NST):
                po = pbig.tile([128, 1024], F32, tag="big", name="po")
                nc.tensor.matmul(po[:, 0:512], aT[:, st, :], wo_sb[:, 0:512])
                nc.tensor.matmul(po[:, 512:1024], aT[:, st, :],
                                 wo_sb[:, 512:1024])
                if st % 2 == 0:
                    nc.scalar.copy(ob[:, st, :], po[:])
                else:
                    nc.vector.tensor_copy(ob[:, st, :], po[:])
            dma(out=g['outp'][:][b], in_=ob[:])


def build_kernel(repeat=1, loopn=0):
    key = (repeat, loopn, NOCOLL, PHASES)
    if key in _CACHE:
        return _CACHE[key]
    nc = bacc.Bacc()
    io = {}
    def din(name, shape, dt):
        io[name] = nc.dram_tensor(name, list(shape), dt, kind="ExternalInput")
    din('xt16', (NST, 128, B, 8, 128), F16)
    din('wqkv16', (8, 4, 128, 128), F16)
    din('wcatT', (NST, 4, 4, 128, 128), F16)
    din('wws', (5, 128, 128), F16)
    din('wpb', (10, 128, 128), F16)
    din('selsum', (24, 128, 18), F16)
    din('ropetabs', (4, NST, 128, 128), F16)
    din('c1_12', (128, NST, 12), F32)
    din('c2it48', (48,), F32)
    din('itau3', (3,), F32)
    din('maskm', (128, 128), BF16)
    din('selqk', (48, 128), F16)
    din('selv48', (48, 128), F16)
    din('bmask', (48, B), F16)
    din('wo16', (128, 1024), F16)
    io['outp'] = nc.dram_tensor('outp', [B, 128, NST, 1024], F16,
                                kind="ExternalOutput")
    nc._kernel_io = io
    from contextlib import ExitStack
    with tile.TileContext(nc) as tc:
        if loopn:
            with tc.For_i(0, loopn):
                with ExitStack() as ctx:
                    _emit(nc, tc, ctx, 0)
        else:
            for rep in range(repeat):
                with ExitStack() as ctx:
                    _emit(nc, tc, ctx, rep)
    nc.finalize()
    _CACHE[key] = nc
    return nc


def prep_inputs(inputs):
    f = np.float32
    x = np.asarray(inputs['x'], f)
    xr8 = np.asarray(x.transpose(2, 0, 1), np.float16).reshape(8, 128, B, NST, 128)
    xt16 = np.ascontiguousarray(xr8.transpose(3, 1, 2, 0, 4))

    wq = np.asarray(inputs['wq'], f)
    wk = np.asarray(inputs['wk'], f)
    wv = np.asarray(inputs['wv'], f)
    wo = np.asarray(inputs['wo'], f)

    pr = {}
    for p, Of in [('q', H * HD), ('k', KVH * HD), ('v', KVH * HD)]:
        A = np.asarray(inputs[f'A_{p}'], f)
        Bm = np.asarray(inputs[f'B_{p}'], f)
        gg = np.asarray(inputs[f'g_{p}'], f)
        bb = np.asarray(inputs[f'b_{p}'], f)
        We = np.asarray(inputs[f'We_{p}'], f)
        tau = float(np.asarray(inputs[f'tau_{p}']))
        itau = 1.0 / max(tau, 1e-6)
        Acat = np.ascontiguousarray(A.transpose(1, 0, 2).reshape(D, NE * R))
        Bp = SCALING * Bm
        Bflat = Bp.reshape(NE * R, Of)
        gv = gg.reshape(NE, Of)
        Wgf = We.reshape(S, NE, Of, NE) * gv[None, :, :, None]
        Wgb = Wgf.sum(axis=1)
        W2 = np.einsum('ero,seoE->serE', Bp, Wgf).reshape(S, NE * R, NE) / NC
        b2 = Bp.sum(axis=2).reshape(NE * R) / NC
        C1 = Wgf.sum(axis=(1, 2)) * itau
        C2 = (We.reshape(S, NE * Of, NE) * bb[None, :, None]).sum((0, 1)) * itau
        pr[p] = dict(Acat=Acat, Bp=Bp, Bflat=Bflat, Wgb=Wgb, W2=W2, b2=b2,
                     C1=C1, C2=C2, itau=itau)

    c1_12 = np.ascontiguousarray(
        np.concatenate([pr[p]['C1'] for p in 'qkv'], 1).astype(f)
        .reshape(NST, 128, 12).transpose(1, 0, 2))
    c2it48 = np.zeros(48, f)
    for pi, p in enumerate('qkv'):
        for b in range(B):
            c2it48[pi * 16 + b * 4:pi * 16 + b * 4 + 4] = pr[p]['C2']
    itau3 = np.array([pr[p]['itau'] for p in 'qkv'], f)
    maskm = np.ascontiguousarray(
        (np.asarray(inputs['mask'], f)[0:128, 0:128].T == 0.0)
        .astype(ml_bf16()))

    cos = np.asarray(inputs['cos'], f)
    sin = np.asarray(inputs['sin'], f)
    Cfull = np.zeros((64, S), f)
    Sfull = np.zeros((64, S), f)
    for i in range(32):
        Cfull[2 * i] = cos[:, i]
        Cfull[2 * i + 1] = cos[:, i]
        Sfull[2 * i] = -sin[:, i]
        Sfull[2 * i + 1] = sin[:, i]
    CA = np.concatenate([Cfull, Cfull], 0) * 0.125
    SA = np.concatenate([Sfull, Sfull], 0) * 0.125
    CB = np.concatenate([Cfull, Cfull], 0)
    SB = np.concatenate([Sfull, Sfull], 0)
    ropetabs = np.stack([CA, SA, CB, SB], 0).reshape(4, 128, NST, 128)
    ropetabs = np.ascontiguousarray(ropetabs.transpose(0, 2, 1, 3)).astype(np.float16)

    selqk = np.zeros((48, 128), np.float16)
    for er in range(128):
        pi, e = er // 64, (er % 64) // 16
        for b in range(B):
            selqk[pi * 16 + b * 4 + e, er] = 1.0
    selv48 = np.zeros((48, 128), np.float16)
    for er in range(64):
        for b in range(B):
            selv48[2 * 16 + b * 4 + er // 16, 64 + er] = 1.0
    bmask = np.zeros((48, B), np.float16)
    for fl in range(48):
        bmask[fl, (fl % 16) // 4] = 1.0

    P = np.zeros((128, 128), f)
    for m in range(128):
        P[m ^ 1, m] = 1.0

    in_maps = []
    for c in range(NC):
        qs = slice(128 * c, 128 * c + 128)
        ks = slice(64 * c, 64 * c + 64)
        mcols = [wq[:, qs],
                 np.concatenate([pr['q']['Acat'],
                                 np.zeros((D, 64), f)], 1),
                 np.concatenate([wk[:, ks], pr['k']['Acat']], 1),
                 np.concatenate([wv[:, ks], pr['v']['Acat']], 1)]
        wqkv16 = np.stack([m.reshape(8, 128, 128) for m in mcols], 1)
        wqkv16 = np.ascontiguousarray(wqkv16.astype(np.float16))

        pi_c = [np.full(128, 0),
                np.concatenate([np.full(64, 0), np.full(64, -1)]),
                np.full(128, 1), np.full(128, 2)]
        selsum = np.zeros((24, 128, 18), np.float16)
        for e in range(4):
            for ci in range(4):
                for frow in range(128):
                    pi = pi_c[ci][frow]
                    if pi >= 0:
                        selsum[e * 4 + ci, frow, 3 * e + pi] = 1.0
        # s2/s1 rows pre-scaled by 1/Ff so stats arrive as E[x^2] and mu
        iF = [1.0 / (NE * H * HD), 1.0 / (NE * KVH * HD), 1.0 / (NE * KVH * HD)]
        for ci in range(4):
            for frow in range(128):
                pi = pi_c[ci][frow]
                if pi >= 0:
                    selsum[16 + ci, frow, 12 + pi] = iF[pi]
        selsum[20, :, 15] = 4.0 * iF[0]
        selsum[21, 0:64, 15] = (pr['q']['b2'] * iF[0]).astype(np.float16)
        selsum[22, 0:64, 16] = 4.0 * iF[1]
        selsum[22, 64:128, 16] = (pr['k']['b2'] * iF[1]).astype(np.float16)
        selsum[23, 0:64, 17] = 4.0 * iF[2]
        selsum[23, 64:128, 17] = (pr['v']['b2'] * iF[2]).astype(np.float16)

        wcatT = np.zeros((NST, 4, 4, 128, 128), np.float16)
        for e in range(4):
            w0 = pr['q']['Wgb'][:, qs, e].T
            w1 = np.zeros((128, S), f)
            w1[0:64] = pr['q']['W2'][:, :, e].T
            w2 = np.zeros((128, S), f)
            w2[0:64] = pr['k']['Wgb'][:, ks, e].T
            w2[64:128] = pr['k']['W2'][:, :, e].T
            w3 = np.zeros((128, S), f)
            w3[0:64] = pr['v']['Wgb'][:, ks, e].T
            w3[64:128] = pr['v']['W2'][:, :, e].T
            for ci, w in enumerate([w0, w1, w2, w3]):
                wcatT[:, e, ci] = w.reshape(128, NST, 128).transpose(1, 0, 2)

        WW0a = (4.0 * np.eye(128, dtype=f)).astype(np.float16)
        t = np.zeros((128, 128), f)
        t[0:64] = 2.0 * pr['q']['Bflat'][:, qs]
        WW0b = t.astype(np.float16)
        Gq = np.zeros((64, 64), f)
        Gk = np.zeros((64, 64), f)
        Gv = np.zeros((64, 64), f)
        for e in range(NE):
            bq = pr['q']['Bp'][e][:, qs]
            bk = pr['k']['Bp'][e][:, ks]
            bvv = pr['v']['Bp'][e][:, ks]
            Gq[e * 16:(e + 1) * 16, e * 16:(e + 1) * 16] = bq @ bq.T
            Gk[e * 16:(e + 1) * 16, e * 16:(e + 1) * 16] = bk @ bk.T
            Gv[e * 16:(e + 1) * 16, e * 16:(e + 1) * 16] = bvv @ bvv.T
        t = np.zeros((128, 128), f)
        t[0:64, 0:64] = Gq
        WW1 = t.astype(np.float16)
        t = np.zeros((128, 128), f)
        t[0:64, 0:64] = 4.0 * np.eye(64)
        t[64:128, 0:64] = 2.0 * pr['k']['Bflat'][:, ks]
        t[64:128, 64:128] = Gk
        WW2 = t.astype(np.float16)
        t = np.zeros((128, 128), f)
        t[0:64, 0:64] = 4.0 * np.eye(64)
        t[64:128, 0:64] = 2.0 * pr['v']['Bflat'][:, ks]
        t[64:128, 64:128] = Gv
        WW3 = t.astype(np.float16)
        wws = np.stack([WW0a, WW0b, WW1, WW2, WW3], 0)

        def pad(a, r0, c0):
            t = np.zeros((128, 128), f)
            t[r0:r0 + a.shape[0], c0:c0 + a.shape[1]] = a
            return t
        WA_base = 2.0 * np.eye(128, dtype=f)
        WA_lora = pad(pr['q']['Bflat'][:, qs], 0, 0)
        WB_base = pad(2.0 * np.eye(64), 0, 0) + pad(2.0 * np.eye(64), 0, 64)
        WB_lora = (pad(pr['k']['Bflat'][:, ks], 64, 0)
                   + pad(pr['k']['Bflat'][:, ks], 64, 64))
        WC_base = pad(2.0 * np.eye(64), 0, 0)
        WC_lora = pad(pr['v']['Bflat'][:, ks], 64, 0)
        wpb = np.stack([WA_base, WA_lora, WA_base @ P, WA_lora @ P,
                        WB_base, WB_lora, WB_base @ P, WB_lora @ P,
                        WC_base, WC_lora], 0).astype(np.float16)

        m = dict(xt16=xt16, wqkv16=wqkv16, wcatT=wcatT, wws=wws, wpb=wpb,
                 selsum=selsum, ropetabs=ropetabs, c1_12=c1_12, c2it48=c2it48,
                 itau3=itau3, maskm=maskm, selqk=selqk, selv48=selv48,
                 bmask=bmask, wo16=wo[qs, :].astype(np.float16))
        in_maps.append(m)
    return in_maps


def run_on_device(in_maps, repeat=1, loopn=0):
    from concourse.bass_utils import run_bass_kernel_spmd
    nc = build_kernel(repeat, loopn)
    res = run_bass_kernel_spmd(nc, in_maps, list(range(NC)))
    return res


def _run_sim(in_maps):
    from concourse.bass_interp import MultiCoreSim
    nc = build_kernel(1)
    sim = MultiCoreSim(nc, NC, num_workers=NC)
    for c in range(NC):
        for name, arr in in_maps[c].items():
            sim.cores[c].tensor(name)[:] = arr
    sim.simulate()
    return [{'outp': np.asarray(sim.cores[c].tensor('outp'))} for c in range(NC)]


def kernel(**inputs):
    in_maps = prep_inputs(inputs)
    try:
        results = run_on_device(in_maps, repeat=1).results
    except Exception as e:
        sys.stderr.write(f"device run failed ({e}); falling back to CoreSim\n")
        results = _run_sim(in_maps)
    out = np.zeros((B, 128, NST, 1024), np.float32)
    for c in range(NC):
        out += np.asarray(results[c]['outp'], np.float32)
    return np.ascontiguousarray(out.transpose(0, 2, 1, 3)).reshape(B, S, 1024)
